# revision 1
# baseline (speedup 1.0000x reference)
"""Causal self-attention (QK-RMSNorm + rotary, H=16, D=1024, B=2, T=2048) on 8 NeuronCores.

Sharding: core c handles batch b = c // 4 and heads 4*(c%4) .. 4*(c%4)+3.
Each core computes the qkv projection for its heads, causal attention, and a
row-parallel slice of the output projection; the host sums the 4 partial
outputs per batch element.

Device layout is feature-major ([dim, token]); the host pre-transposes x and
the weight slices so no on-chip transposes of x are needed. Rotary is applied
via a second projection against sign-permuted weight columns (rot(q) = x @
W_rot), so all vector ops stay partition-aligned. Softmax needs no running
max: RMS-normalized q,k bound scores to |s| <= sqrt(d_head) * ||q|| = 8.

All matmuls keep K=128 and N>=256 (fp32r fast path): q is stored zero-padded
per head so scores contract the full 128 partitions; v blocks are sliced 128
wide (trailing columns are don't-care rows in PSUM); y is written in head
pairs, with odd heads reading the previous v-block's ones column so their
softmax sum lands on a partition the pair layout can use.
"""
import sys
sys.path.insert(0, '/opt/trn_rl_repo')

import numpy as np
from contextlib import ExitStack

import concourse.bass as bass
import concourse.tile as tile
from concourse import bacc, mybir
from concourse.bass_utils import run_bass_kernel_spmd

F32 = mybir.dt.float32
F32R = mybir.dt.float32r
AF = mybir.ActivationFunctionType

N_HEAD = 16
D_MODEL = 1024
D_HEAD = 64
B, T = 2, 2048
N_CORES = 8
HL = 4            # heads per core
KT = D_MODEL // 128   # 8 contraction tiles
NCH = T // 512    # 4 t-chunks per core
NIB = T // 512    # 4 i-blocks
NTT = T // 128    # 16 t-tiles
SCALE = D_HEAD ** -0.5

_cached = {}


def _build():
    nc = bacc.Bacc("TRN2", target_bir_lowering=False, debug=False,
                   num_devices=N_CORES)

    # ---- DRAM I/O ----------------------------------------------------------
    xT = nc.dram_tensor("xT", [D_MODEL, T], F32R, kind="ExternalInput").ap()
    wqk = nc.dram_tensor("wqk", [D_MODEL, 512], F32R, kind="ExternalInput").ap()
    wqkr = nc.dram_tensor("wqkr", [D_MODEL, 512], F32R, kind="ExternalInput").ap()
    wv = nc.dram_tensor("wv", [D_MODEL, 256], F32R, kind="ExternalInput").ap()
    wpP = nc.dram_tensor("wpP", [2, 128, 1024], F32R, kind="ExternalInput").ap()
    cosT = nc.dram_tensor("cosT", [128, T], F32, kind="ExternalInput").ap()
    sinT = nc.dram_tensor("sinT", [128, T], F32, kind="ExternalInput").ap()
    trimask = nc.dram_tensor("trimask", [128, 128], F32R, kind="ExternalInput").ap()
    rsel32 = nc.dram_tensor("rsel32", [128, 16 * 32], F32R,
                            kind="ExternalInput").ap()
    rselT32 = nc.dram_tensor("rselT32", [32, 16 * 128], F32R,
                             kind="ExternalInput").ap()
    sel16p = nc.dram_tensor("sel16p", [16, 8 * 128], F32R,
                            kind="ExternalInput").ap()
    onescol = nc.dram_tensor("onescol", [128, HL * NTT], F32R,
                             kind="ExternalInput").ap()
    zpad = nc.dram_tensor("zpad", [64, T], F32R, kind="ExternalInput").ap()
    ident = nc.dram_tensor("ident", [128, 128], F32R, kind="ExternalInput").ap()
    out = nc.dram_tensor("out", [T, D_MODEL], F32, kind="ExternalOutput").ap()

    with tile.TileContext(nc) as tc, ExitStack() as ctx:
        ctx.enter_context(nc.allow_low_precision(
            reason="float32r tiles share fp32 bit layout; matmul runs fp32r"))

        cpool = ctx.enter_context(tc.tile_pool(name="consts", bufs=1))
        ppool = ctx.enter_context(tc.tile_pool(name="persist", bufs=1))
        epool = ctx.enter_context(tc.tile_pool(name="exps", bufs=3))
        ps_a = ctx.enter_context(tc.tile_pool(name="psa", bufs=2, space="PSUM"))
        ps_b = ctx.enter_context(tc.tile_pool(name="psb", bufs=2, space="PSUM"))
        ps_c = ctx.enter_context(tc.tile_pool(name="psc", bufs=2, space="PSUM"))
        ps_y = ctx.enter_context(tc.tile_pool(name="psy", bufs=2, space="PSUM"))

        tri_sb = cpool.tile([128, 128], F32R)
        rsel32_sb = cpool.tile([128, 16 * 32], F32R)
        nc.sync.dma_start(rsel32_sb[:], rsel32[:])
        rselT32_sb = cpool.tile([32, 16 * 128], F32R)
        nc.sync.dma_start(rselT32_sb[:], rselT32[:])
        sel16p_sb = cpool.tile([16, 8 * 128], F32R)
        id_sb = cpool.tile([128, 128], F32R)
        nc.sync.dma_start(id_sb[:], ident[:])

        # persistent activations: zero-padded per-head q, paired k, v blocks
        # of [64 dims | ones] with 64 pad columns at the end, paired y
        qTz = [ppool.tile([128, T], F32R, tag=f"qTz{h}", name=f"qTz{h}")
               for h in range(HL)]
        kT_sb = [ppool.tile([128, T], F32R, tag=f"kT{t}", name=f"kT{t}")
                 for t in range(2)]
        v_sb = ppool.tile([128, HL * NTT * 65 + 64], F32R, tag="v")
        v_blk = v_sb[:, 0:HL * NTT * 65].rearrange("p (g o) -> p g o", o=65)
        yP = [ppool.tile([128, T], F32R, tag=f"yP{t}", name=f"yP{t}")
              for t in range(2)]

        # ---- phase 1: projections + rmsnorm + rope + v transpose -----------
        with tc.tile_pool(name="wts", bufs=1) as wtp, \
             tc.tile_pool(name="xtp", bufs=2) as xpool, \
             tc.tile_pool(name="pwork", bufs=2) as wpool:
            wqk_sb = wtp.tile([128, KT, 512], F32R)
            wqkr_sb = wtp.tile([128, KT, 512], F32R)
            wv_sb = wtp.tile([128, KT, 256], F32R)
            xt0 = None
            for k in range(KT):
                ks = slice(k * 128, (k + 1) * 128)
                if k == 0:
                    xt0 = xpool.tile([128, KT, 512], F32R, tag="xt", name="xt0")
                nc.sync.dma_start(xt0[:, k, :], xT[ks, 0:512])
                nc.sync.dma_start(wqk_sb[:, k, :], wqk[ks, :])
                nc.sync.dma_start(wqkr_sb[:, k, :], wqkr[ks, :])
                nc.sync.dma_start(wv_sb[:, k, :], wv[ks, :])
            cos_sb = wtp.tile([128, T], F32)
            nc.sync.dma_start(cos_sb[:], cosT[:])
            sin_sb = wtp.tile([128, T], F32)
            nc.sync.dma_start(sin_sb[:], sinT[:])
            ssq_all = ps_c.tile([32, 512], F32, tag="stat", name="ssq_all")

            for ch in range(NCH):
                cs = ch * 512
                if ch == 0:
                    xt = xt0
                else:
                    xt = xpool.tile([128, KT, 512], F32R, tag="xt")
                    for k in range(KT):
                        nc.sync.dma_start(xt[:, k, :],
                                          xT[k * 128:(k + 1) * 128, cs:cs + 512])

                # q/k M-tiles: 0,1 = q head-pairs; 2,3 = k head-pairs
                for mt in range(4):
                    acc = ps_a.tile([128, 512], F32, tag="qk")
                    accr = ps_b.tile([128, 512], F32, tag="qkr")
                    for k in range(KT):
                        nc.tensor.matmul(acc[:],
                                         wqk_sb[:, k, mt * 128:(mt + 1) * 128],
                                         xt[:, k, :], start=(k == 0),
                                         stop=(k == KT - 1))
                    for k in range(KT):
                        nc.tensor.matmul(accr[:],
                                         wqkr_sb[:, k, mt * 128:(mt + 1) * 128],
                                         xt[:, k, :], start=(k == 0),
                                         stop=(k == KT - 1))
                    t1 = wpool.tile([128, 512], F32, tag="t1")
                    nc.vector.tensor_mul(t1[:], acc[:], cos_sb[:, cs:cs + 512])
                    t2 = wpool.tile([128, 512], F32, tag="t2")
                    nc.vector.tensor_mul(t2[:], accr[:], sin_sb[:, cs:cs + 512])
                    # rope output written unscaled; rms scale applied in-place
                    # after the batched ln/exp pass (one ACT table set swap)
                    if mt < 2:
                        dsts = [qTz[2 * mt][0:64, cs:cs + 512],
                                qTz[2 * mt + 1][64:128, cs:cs + 512]]
                        nc.vector.tensor_add(dsts[0], t1[0:64, :], t2[0:64, :])
                        nc.vector.tensor_add(dsts[1], t1[64:128, :],
                                             t2[64:128, :])
                    else:
                        dsts = [kT_sb[mt - 2][:, cs:cs + 512]]
                        nc.vector.tensor_add(dsts[0], t1[:], t2[:])
                    sq = wpool.tile([128, 512], F32R, tag="sq")
                    for d in dsts:
                        b0 = d.base_partition() if callable(d.base_partition) \
                            else d.base_partition
                        nc.scalar.square(sq[b0:b0 + d.shape[0], :], d)
                    idx = ch * 4 + mt
                    nc.tensor.matmul(ssq_all[:],
                                     rsel32_sb[:, idx * 32:(idx + 1) * 32],
                                     sq[:], start=(idx == 0), stop=(idx == 15))

                # v: project feature-major, then transpose to token-major
                for mt in range(2):
                    accv = ps_a.tile([128, 512], F32, tag="qk")
                    for k in range(KT):
                        nc.tensor.matmul(accv[:],
                                         wv_sb[:, k, mt * 128:(mt + 1) * 128],
                                         xt[:, k, :], start=(k == 0),
                                         stop=(k == KT - 1))
                    vtc = wpool.tile([128, 512], F32R, tag="vtc")
                    nc.scalar.copy(vtc[:], accv[:])
                    tps = ps_b.tile([128, 512], F32R, tag="qkr")
                    for s in range(4):
                        nc.tensor.transpose(tps[:, s * 128:(s + 1) * 128],
                                            vtc[:, s * 128:(s + 1) * 128],
                                            id_sb[:])
                    o = tps[:].rearrange("p (s h d) -> p s h d", s=4, h=2)
                    for h2 in range(2):
                        h = mt * 2 + h2
                        dst = v_blk[:, h * NTT + ch * 4:h * NTT + ch * 4 + 4,
                                    0:64]
                        nc.scalar.copy(dst, o[:, :, h2, :])

            # batched rsqrt = exp(-0.5 ln(ms)) over all 32 (tile, half) rows
            lnv_all = wpool.tile([32, 512], F32, tag="t1")
            nc.scalar.activation(lnv_all[:], ssq_all[:], AF.Ln, scale=1.0 / 64.0)
            rms_all = wpool.tile([32, 512], F32R, tag="t2")
            nc.scalar.activation(rms_all[:], lnv_all[:], AF.Exp, scale=-0.5)
            for ch in range(NCH):
                cs = ch * 512
                for mt in range(4):
                    idx = ch * 4 + mt
                    bc = ps_c.tile([128, 512], F32, tag="stat")
                    nc.tensor.matmul(bc[:],
                                     rselT32_sb[:, idx * 128:(idx + 1) * 128],
                                     rms_all[:], start=True, stop=True)
                    if mt < 2:
                        dsts = [qTz[2 * mt][0:64, cs:cs + 512],
                                qTz[2 * mt + 1][64:128, cs:cs + 512]]
                    else:
                        dsts = [kT_sb[mt - 2][:, cs:cs + 512]]
                    for d in dsts:
                        b0 = d.base_partition() if callable(d.base_partition) \
                            else d.base_partition
                        nc.vector.tensor_mul(d, d, bc[b0:b0 + d.shape[0], :])

        # deferred constant loads (not needed until attention)
        for h in range(HL):
            half = slice(64, 128) if h % 2 == 0 else slice(0, 64)
            nc.sync.dma_start(qTz[h][half, :], zpad[:])
        nc.sync.dma_start(v_blk[:, :, 64:65], onescol.unsqueeze(2))
        nc.sync.dma_start(tri_sb[:], trimask[:])
        nc.sync.dma_start(sel16p_sb[:], sel16p[:])

        # ---- phase 2: attention, batched softmax division, out proj --------
        with tc.tile_pool(name="wpp", bufs=1) as wpp, \
             tc.tile_pool(name="ysg", bufs=1) as ysgp, \
             tc.tile_pool(name="awork", bufs=2) as awork:
            wpP_sb = [wpp.tile([128, 1024], F32R, tag=f"wpP{t}", name=f"wpP{t}")
                      for t in range(2)]
            for t in range(2):
                nc.sync.dma_start(wpP_sb[t][:], wpP[t])
            sums_all = wpp.tile([16, 512], F32, tag="sums")
            ySG = [ysgp.tile([128, 512], F32, tag=f"ySG{r}", name=f"ySG{r}")
                   for r in range(16)]

            for h in range(HL):
                ht, hh = h // 2, h % 2
                for ib in range(NIB):
                    r = h * NIB + ib
                    ibs = ib * 512
                    njt = 4 * (ib + 1)
                    yacc = ps_y.tile([128, 512], F32, tag="yacc")
                    for jt in range(njt):
                        o = max(0, jt * 128 - ibs)
                        w = 512 - o
                        s_ps = ps_b.tile([128, 512], F32, tag="qkr")
                        nc.tensor.matmul(s_ps[:, 0:w],
                                         kT_sb[ht][:, jt * 128:(jt + 1) * 128],
                                         qTz[h][:, ibs + o:ibs + 512],
                                         start=True, stop=True)
                        p_sb = epool.tile([128, 512], F32R, tag="p")
                        nc.scalar.activation(p_sb[:, 0:w], s_ps[:, 0:w], AF.Exp,
                                             scale=SCALE)
                        if jt * 128 >= ibs:  # diagonal tile: triangular mask
                            nc.vector.tensor_mul(p_sb[:, 0:128], p_sb[:, 0:128],
                                                 tri_sb[:])
                        g = h * NTT + jt
                        if hh == 0:
                            vau = v_sb[:, g * 65:g * 65 + 128]  # y@0-63, sum@64
                        else:
                            vau = v_sb[:, g * 65 - 64:g * 65 + 64]  # sum@63, y@64+
                        nc.tensor.matmul(yacc[:, o:512], vau, p_sb[:, 0:w],
                                         start=(jt == 0), stop=(jt == njt - 1))
                    # stage y + its softmax sums to SBUF; collect sums by DMA
                    nc.scalar.copy(ySG[r][:], yacc[:])
                    srow = 64 if hh == 0 else 63
                    nc.sync.dma_start(sums_all[r:r + 1, :],
                                      ySG[r][srow:srow + 1, :])

            # batched softmax division into paired y
            recip_all = awork.tile([16, 512], F32R, tag="recip")
            nc.vector.reciprocal(recip_all[:], sums_all[:])
            for hp in range(2):
                for ib in range(NIB):
                    bc2 = ps_c.tile([128, 512], F32, tag="stat")
                    nc.tensor.matmul(
                        bc2[:], sel16p_sb[:, (hp * NIB + ib) * 128:
                                          (hp * NIB + ib + 1) * 128],
                        recip_all[:], start=True, stop=True)
                    re, ro = 2 * hp * NIB + ib, (2 * hp + 1) * NIB + ib
                    ibs = ib * 512
                    nc.vector.tensor_mul(yP[hp][0:64, ibs:ibs + 512],
                                         ySG[re][0:64, :], bc2[0:64, :])
                    nc.vector.tensor_mul(yP[hp][64:128, ibs:ibs + 512],
                                         ySG[ro][64:128, :], bc2[64:128, :])

            # output projection (row-parallel over this core's head dims)
            for mt in range(NTT):
                for oc in range(2):
                    acc = ps_a.tile([128, 512], F32, tag="qk")
                    for t in range(2):
                        nc.tensor.matmul(acc[:],
                                         yP[t][:, mt * 128:(mt + 1) * 128],
                                         wpP_sb[t][:, oc * 512:(oc + 1) * 512],
                                         start=(t == 0), stop=(t == 1))
                    o_sb = awork.tile([128, 512], F32, tag="osb")
                    nc.vector.tensor_copy(o_sb[:], acc[:])
                    nc.sync.dma_start(out[mt * 128:(mt + 1) * 128,
                                          oc * 512:(oc + 1) * 512], o_sb[:])

    nc.compile()
    return nc


def _host_inputs(x, w_attn, w_proj):
    """Build the 8 per-core input maps."""
    inv_freq = 1.0 / (10000.0 ** (np.arange(0, D_HEAD, 2, dtype=np.float32)
                                  / D_HEAD))
    t = np.arange(T, dtype=np.float32)
    freqs = np.einsum('i,j->ij', t, inv_freq)          # [T, 32]
    cos64 = np.cos(np.concatenate([freqs, freqs], 1)).T  # [64, T]
    sin64 = np.sin(np.concatenate([freqs, freqs], 1)).T
    cosT = np.concatenate([cos64, cos64], 0).astype(np.float32)  # [128, T]
    sinT = np.concatenate([sin64, sin64], 0).astype(np.float32)

    tri = (np.arange(128)[:, None] <= np.arange(128)[None, :]).astype(np.float32)
    rsel32 = np.zeros((128, 16 * 32), np.float32)
    rselT32 = np.zeros((32, 16 * 128), np.float32)
    for chm in range(16):
        ch, mt = chm // 4, chm % 4
        for half in range(2):
            r = ch * 8 + mt * 2 + half
            ps = slice(half * 64, half * 64 + 64)
            rsel32[ps, chm * 32 + r] = 1.0
            rselT32[r, chm * 128 + half * 64:chm * 128 + half * 64 + 64] = 1.0
    # sel16p[(hp,ib) block]: rows 0-63 pick sums row of even head, 64-127 odd
    sel16p = np.zeros((16, 8 * 128), np.float32)
    for hp in range(2):
        for ib in range(NIB):
            blk = (hp * NIB + ib) * 128
            sel16p[(2 * hp) * NIB + ib, blk:blk + 64] = 1.0
            sel16p[(2 * hp + 1) * NIB + ib, blk + 64:blk + 128] = 1.0
    ident = np.eye(128, dtype=np.float32)
    onescol = np.ones((128, HL * NTT), np.float32)
    zpad = np.zeros((64, T), np.float32)

    wq = w_attn[:D_MODEL]          # [1024, 1024] rows: head h = 64h..64h+63
    wk = w_attn[D_MODEL:2 * D_MODEL]
    wv_full = w_attn[2 * D_MODEL:]

    def rot_rows(w):
        # rows of w are per-head output dims; rot(q)[d] = -q[d+32] / q[d-32]
        w = w.reshape(N_HEAD, D_HEAD, D_MODEL)
        wr = np.concatenate([-w[:, 32:, :], w[:, :32, :]], axis=1)
        return wr.reshape(N_HEAD * D_HEAD, D_MODEL)

    wqr_full = rot_rows(wq)
    wkr_full = rot_rows(wk)

    in_maps = []
    for c in range(N_CORES):
        b, hg = c // 4, c % 4
        hs = slice(hg * 4 * D_HEAD, (hg * 4 + 4) * D_HEAD)   # 256 rows
        wqk_c = np.concatenate([wq[hs], wk[hs]], 0).T.copy()       # [1024, 512]
        wqkr_c = np.concatenate([wqr_full[hs], wkr_full[hs]], 0).T.copy()
        wv_c = wv_full[hs].T.copy()                                # [1024, 256]
        wp_c = [w_proj[:, (hg * 4 + j) * D_HEAD:(hg * 4 + j + 1) * D_HEAD].T
                for j in range(HL)]                                # 4x[64,1024]
        wpP_c = np.stack([np.concatenate([wp_c[0], wp_c[1]], 0),
                          np.concatenate([wp_c[2], wp_c[3]], 0)])  # [2,128,1024]
        in_maps.append({
            "xT": np.ascontiguousarray(x[b].T),
            "wqk": np.ascontiguousarray(wqk_c),
            "wqkr": np.ascontiguousarray(wqkr_c),
            "wv": np.ascontiguousarray(wv_c),
            "wpP": np.ascontiguousarray(wpP_c),
            "cosT": cosT, "sinT": sinT, "trimask": tri,
            "rsel32": rsel32, "rselT32": rselT32,
            "sel16p": sel16p, "onescol": onescol, "zpad": zpad,
            "ident": ident,
        })
    return in_maps


def kernel(x, w_attn, w_proj, _want_results=False):
    x = np.asarray(x, dtype=np.float32)
    w_attn = np.asarray(w_attn, dtype=np.float32)
    w_proj = np.asarray(w_proj, dtype=np.float32)

    if "nc" not in _cached:
        _cached["nc"] = _build()
    nc = _cached["nc"]

    in_maps = _host_inputs(x, w_attn, w_proj)
    res = run_bass_kernel_spmd(nc, in_maps, list(range(N_CORES)))

    full = np.zeros((B, T, D_MODEL), np.float32)
    for c in range(N_CORES):
        full[c // 4] += res.results[c]["out"]
    if _want_results:
        return full, res
    return full



# revision 19
# speedup vs baseline: 1.0571x; 1.0571x over previous
"""Causal self-attention (QK-RMSNorm + rotary, H=16, D=1024, B=2, T=2048) on 8 NeuronCores.

Sharding: core c handles batch b = c // 4 and heads 4*(c%4) .. 4*(c%4)+3,
processed as two head PAIRS. Each core computes the qkv projection for its
heads, causal attention, and a row-parallel slice of the output projection;
the host sums the 4 partial outputs per batch element.

v2 design (vs the fp32r baseline):
- All matmul operands in bf16 (x, weights, q, k, v, p, y): halves DMA/SBUF.
- Rotary via a cross-partition DMA shift (rot(q) = shift(q) * signed-sin)
  instead of a second full projection: saves ~57k PE cycles.
- RMS scale applied to raw q/k before rope (rope is norm-preserving and
  commutes with per-head scalars); stats batched into one ln+exp per pair.
- Attention: full-512-wide score matmuls, exp batched in [128,1024] pairs
  (amortizes ACT's per-instruction bubble), causal masking via a Pool-side
  tri multiply on the diagonal squares, y-matmuls restricted to [o:512].
- Softmax sums ride the v ones-column (partition 64/63 of yacc) as in the
  baseline; division is per i-block so the out-projection overlaps the tail
  of attention.
- Engine balance: PE does matmuls only; ACT does exps+stats; DVE does
  PSUM->SBUF copies and PSUM-operand muls; Pool (gpsimd) does SBUF-only
  muls/adds (rope combine, tri); DMA does the rotary shift and v transpose.
"""
import sys
sys.path.insert(0, '/opt/trn_rl_repo')

import numpy as np
import ml_dtypes
from contextlib import ExitStack

import concourse.bass as bass
import concourse.tile as tile
from concourse import bacc, mybir
from concourse.bass_utils import run_bass_kernel_spmd

F32 = mybir.dt.float32
BF = mybir.dt.bfloat16
AF = mybir.ActivationFunctionType

N_HEAD = 16
D_MODEL = 1024
D_HEAD = 64
B, T = 2, 2048
N_CORES = 8
HL = 4              # heads per core
KT = D_MODEL // 128  # 8 contraction tiles
NCH = T // 512      # 4 token chunks
NIB = T // 512      # 4 i-blocks
NTT = T // 128      # 16 j-tiles
SCALE = D_HEAD ** -0.5

_cached = {}


def _build(debug_dump=False):
    nc = bacc.Bacc("TRN2", target_bir_lowering=False, debug=False,
                   num_devices=N_CORES)

    # ---- DRAM I/O ----------------------------------------------------------
    xT = nc.dram_tensor("xT", [D_MODEL, T], BF, kind="ExternalInput").ap()
    wA = nc.dram_tensor("wA", [2, D_MODEL, 384], BF, kind="ExternalInput").ap()
    cosT = nc.dram_tensor("cosT", [128, T], BF, kind="ExternalInput").ap()
    sinNegT = nc.dram_tensor("sinNegT", [128, T], BF,
                             kind="ExternalInput").ap()
    trimask = nc.dram_tensor("trimask", [128, 128], BF,
                             kind="ExternalInput").ap()
    selqk = nc.dram_tensor("selqk", [128, 8, 16], BF,
                           kind="ExternalInput").ap()
    selbc = nc.dram_tensor("selbc", [16, 8, 128], BF,
                           kind="ExternalInput").ap()
    sel2 = nc.dram_tensor("sel2", [2, 128], BF, kind="ExternalInput").ap()
    zpadQ = nc.dram_tensor("zpadQ", [64, T], BF, kind="ExternalInput").ap()
    onescol = nc.dram_tensor("onescol", [128, HL * NTT], BF,
                             kind="ExternalInput").ap()
    wpP = nc.dram_tensor("wpP", [2, 128, 1024], BF, kind="ExternalInput").ap()
    out = nc.dram_tensor("out", [T, D_MODEL], F32, kind="ExternalOutput").ap()
    if debug_dump:
        dbg = {
            "d_qsb": nc.dram_tensor("d_qsb", [2, 128, T], BF,
                                    kind="ExternalOutput").ap(),
            "d_qTz": nc.dram_tensor("d_qTz", [HL, 128, T], BF,
                                    kind="ExternalOutput").ap(),
            "d_kT": nc.dram_tensor("d_kT", [2, 128, T], BF,
                                   kind="ExternalOutput").ap(),
            "d_v": nc.dram_tensor("d_v", [128, 5200], BF,
                                  kind="ExternalOutput").ap(),
            "d_sums": nc.dram_tensor("d_sums", [2, 2, NIB, 512], F32,
                                     kind="ExternalOutput").ap(),
            "d_yP": nc.dram_tensor("d_yP", [2, 128, T], BF,
                                   kind="ExternalOutput").ap(),
        }

    with tile.TileContext(nc) as tc, ExitStack() as ctx:
        ctx.enter_context(nc.allow_low_precision(
            reason="bf16 matmuls/intermediates; tolerance is 2e-2"))

        cpool = ctx.enter_context(tc.tile_pool(name="consts", bufs=1))
        work = ctx.enter_context(tc.tile_pool(name="work", bufs=2))
        ps_s = ctx.enter_context(tc.tile_pool(name="pss", bufs=2,
                                              space="PSUM"))
        ps_y = ctx.enter_context(tc.tile_pool(name="psy", bufs=1,
                                              space="PSUM"))

        # ---- persistent SBUF -----------------------------------------------
        x_sb = cpool.tile([128, KT, T], BF)
        wA_sb = [cpool.tile([128, KT, 384], BF, name=f"wA{p}") for p in (0, 1)]
        cos_sb = cpool.tile([128, T], BF)
        sinNeg_sb = cpool.tile([128, T], BF)
        tri_sb = cpool.tile([128, 128], BF)
        selqk_sb = cpool.tile([128, 8, 16], BF)
        selbc_sb = cpool.tile([16, 8, 128], BF)
        sel2_sb = cpool.tile([2, 128], BF)
        wpP_sb = [cpool.tile([128, 1024], BF, name=f"wpP{p}") for p in (0, 1)]
        qTz = [cpool.tile([128, T], BF, name=f"qTz{h}") for h in range(HL)]
        kT_sb = [cpool.tile([128, T], BF, name=f"kT{p}") for p in (0, 1)]
        # padded past HL*NTT*65+64 so the [p, 2, 1040] transpose-dst view of
        # the last head pair stays in bounds
        v_sb = cpool.tile([128, 5200], BF)
        v3 = v_sb[:, 0:HL * NTT * 65].rearrange("p (g o) -> p g o", o=65)
        yP = [cpool.tile([128, T], BF, name=f"yP{p}") for p in (0, 1)]
        qsbF = [[cpool.tile([128, T], BF, name=f"qsb{p}{m}") for m in (0, 1)]
                for p in (0, 1)]
        qshF = [[cpool.tile([128, T], BF, name=f"qsh{p}{m}") for m in (0, 1)]
                for p in (0, 1)]
        rinv_sb = [cpool.tile([16, 512], BF, name=f"rinv{p}") for p in (0, 1)]
        sums_sb = [cpool.tile([2, NIB, 512], F32, name=f"sums{p}")
                   for p in (0, 1)]
        rinvy_sb = [cpool.tile([2, NIB, 512], BF, name=f"rinvy{p}")
                    for p in (0, 1)]

        # ---- preamble DMAs -------------------------------------------------
        nc.sync.dma_start(wA_sb[0][:],
                          wA[0].rearrange("(k p) c -> p k c", p=128))
        for ch in range(NCH):
            nc.sync.dma_start(x_sb[:, :, ch * 512:(ch + 1) * 512],
                              xT.rearrange("(k p) t -> p k t", p=128)
                              [:, :, ch * 512:(ch + 1) * 512])
        nc.sync.dma_start(selqk_sb[:], selqk[:])
        nc.sync.dma_start(wA_sb[1][:],
                          wA[1].rearrange("(k p) c -> p k c", p=128))
        nc.sync.dma_start(cos_sb[:], cosT[:])
        nc.sync.dma_start(sinNeg_sb[:], sinNegT[:])
        nc.sync.dma_start(tri_sb[:], trimask[:])
        nc.sync.dma_start(selbc_sb[:], selbc[:])
        nc.sync.dma_start(sel2_sb[:], sel2[:])
        for h in range(HL):
            half = slice(64, 128) if h % 2 == 0 else slice(0, 64)
            nc.sync.dma_start(qTz[h][half, :], zpadQ[:])
        nc.sync.dma_start(v3[:, :, 64:65], onescol.unsqueeze(2))
        for p in (0, 1):
            nc.sync.dma_start(wpP_sb[p][:], wpP[p])

        ySG_store = {}

        # ---- unit emitters -------------------------------------------------
        def proj_unit(ps_w, hp, m, ch, st_tile):
            cs = slice(ch * 512, (ch + 1) * 512)
            acc = ps_w.tile([128, 512], F32, tag="pa", bufs=2, name="acc")
            for k in range(KT):
                nc.tensor.matmul(acc[:],
                                 wA_sb[hp][:, k, m * 128:(m + 1) * 128],
                                 x_sb[:, k, cs], start=(k == 0),
                                 stop=(k == KT - 1))
            if m < 2:
                dst = qsbF[hp][m][:, cs]
                nc.vector.tensor_copy(dst, acc[:])
                sqt = work.tile([128, 512], BF, tag="sq", name="sqt")
                nc.gpsimd.tensor_mul(sqt[:], dst, dst)
                idx = m * 4 + ch
                nc.tensor.matmul(st_tile[:], selqk_sb[:, idx, :], sqt[:],
                                 start=(idx == 0), stop=(idx == 7))
            else:
                vdst = work.tile([128, 512], BF, tag="vsb", bufs=3,
                                 name="vdst")
                nc.vector.tensor_copy(vdst[:], acc[:])
                for s4 in range(4):
                    jt = ch * 4 + s4
                    gA = (2 * hp) * NTT + jt
                    vstg = work.tile([128, 128], BF, tag="vstg", bufs=3,
                                     name="vstg")
                    nc.sync.dma_start_transpose(
                        vstg[:], vdst[:, s4 * 128:(s4 + 1) * 128])
                    vv = v_sb[:, gA * 65:gA * 65 + 2080] \
                        .rearrange("p (h x) -> p h x", h=2)[:, :, 0:64]
                    nc.gpsimd.tensor_copy(
                        vv, vstg[:].rearrange("p (h x) -> p h x", h=2))

        def lnexp(hp, st_tile):
            lnt = work.tile([16, 512], F32, tag="lnt", name="lnt")
            nc.scalar.activation(lnt[:], st_tile[:], AF.Ln, scale=1.0 / 64.0)
            nc.scalar.activation(rinv_sb[hp][:], lnt[:], AF.Exp, scale=-0.5)

        def fin_chunks(hp):
            """Chunk closures: rms-scale in place, rotary shift + combine."""
            chunks = []
            for m in (0, 1):
                for ch in range(NCH):
                    def bc_scale(m=m, ch=ch):
                        cs = slice(ch * 512, (ch + 1) * 512)
                        bc = ps_s.tile([128, 512], F32, tag="s", name="bc")
                        nc.tensor.matmul(bc[:], selbc_sb[:, m * 4 + ch, :],
                                         rinv_sb[hp][:], start=True,
                                         stop=True)
                        nc.vector.tensor_mul(qsbF[hp][m][:, cs],
                                             qsbF[hp][m][:, cs], bc[:])
                    chunks.append(bc_scale)

                def shifts(m=m):
                    for blk in range(4):
                        d0 = blk * 32
                        s0 = (blk ^ 1) * 32
                        nc.sync.dma_start(qshF[hp][m][d0:d0 + 32, :],
                                          qsbF[hp][m][s0:s0 + 32, :])
                chunks.append(shifts)
            for m in (0, 1):
                for ch in range(NCH):
                    def rope(m=m, ch=ch):
                        cs = slice(ch * 512, (ch + 1) * 512)
                        t1 = work.tile([128, 512], BF, tag="t1", name="t1")
                        nc.gpsimd.tensor_mul(t1[:], qsbF[hp][m][:, cs],
                                             cos_sb[:, cs])
                        t2 = work.tile([128, 512], BF, tag="t2", name="t2")
                        nc.gpsimd.tensor_mul(t2[:], qshF[hp][m][:, cs],
                                             sinNeg_sb[:, cs])
                        if m == 0:
                            nc.gpsimd.tensor_add(qTz[2 * hp][0:64, cs],
                                                 t1[0:64, :], t2[0:64, :])
                            nc.gpsimd.tensor_add(qTz[2 * hp + 1][64:128, cs],
                                                 t1[64:128, :],
                                                 t2[64:128, :])
                        else:
                            nc.gpsimd.tensor_add(kT_sb[hp][:, cs], t1[:],
                                                 t2[:])
                    chunks.append(rope)
            return chunks

        def attn_unit(hp, hh, ib):
            h_l = 2 * hp + hh
            njt = 4 * (ib + 1)
            ibs = ib * 512
            yacc = ps_y.tile([128, 512], F32, tag="y", name="yacc")

            def ymms(pr, pt):
                for half in (0, 1):
                    jt = 2 * pr + half
                    o = max(0, jt * 128 - ibs)
                    g = h_l * NTT + jt
                    if hh == 0:
                        vau = v_sb[:, g * 65:g * 65 + 128]
                    else:
                        vau = v_sb[:, g * 65 - 64:g * 65 + 64]
                    nc.tensor.matmul(yacc[:, o:512], vau,
                                     pt[:, half * 512 + o:half * 512 + 512],
                                     start=(jt == 0), stop=(jt == njt - 1))

            prev = None
            for pr in range(njt // 2):
                sp = ps_s.tile([128, 1024], F32, tag="s", name="sp")
                for half in (0, 1):
                    jt = 2 * pr + half
                    nc.tensor.matmul(sp[:, half * 512:(half + 1) * 512],
                                     kT_sb[hp][:, jt * 128:(jt + 1) * 128],
                                     qTz[h_l][:, ibs:ibs + 512],
                                     start=True, stop=True)
                pt = work.tile([128, 1024], BF, tag="p", bufs=3, name="pt")
                nc.scalar.activation(pt[:], sp[:], AF.Exp, scale=SCALE)
                for half in (0, 1):
                    jt = 2 * pr + half
                    o = jt * 128 - ibs
                    if o >= 0:
                        lo = half * 512 + o
                        nc.gpsimd.tensor_mul(pt[:, lo:lo + 128],
                                             pt[:, lo:lo + 128], tri_sb[:])
                if prev is not None:
                    ymms(*prev)
                prev = (pr, pt)
            ymms(*prev)

            ySG = work.tile([128, 512], F32, tag="ysg", bufs=10, name="ySG")
            nc.vector.tensor_copy(ySG[:], yacc[:])
            srow = 64 if hh == 0 else 63
            nc.sync.dma_start(sums_sb[hp][hh:hh + 1, ib, :],
                              ySG[srow:srow + 1, :])
            ySG_store[(hp, hh, ib)] = ySG

        def ydiv_unit(ps_o, hp, ib):
            ibs = ib * 512
            nc.vector.reciprocal(rinvy_sb[hp][:, ib, :],
                                 sums_sb[hp][:, ib, :])
            bc2 = ps_o.tile([128, 512], F32, tag="bc2", bufs=1, name="bc2")
            nc.tensor.matmul(bc2[:], sel2_sb[:], rinvy_sb[hp][:, ib, :],
                             start=True, stop=True)
            e = ySG_store[(hp, 0, ib)]
            o_ = ySG_store[(hp, 1, ib)]
            nc.vector.tensor_mul(yP[hp][0:64, ibs:ibs + 512], e[0:64, :],
                                 bc2[0:64, :])
            nc.vector.tensor_mul(yP[hp][64:128, ibs:ibs + 512],
                                 o_[64:128, :], bc2[64:128, :])

        def o_unit(ps_o, ib):
            for mt in range(4 * ib, 4 * ib + 4):
                ost = work.tile([128, 1024], F32, tag="ost", name="ost")
                for oc in (0, 1):
                    oa = ps_o.tile([128, 512], F32, tag="oa", name="oa")
                    for t in (0, 1):
                        nc.tensor.matmul(oa[:],
                                         yP[t][:, mt * 128:(mt + 1) * 128],
                                         wpP_sb[t][:, oc * 512:(oc + 1) * 512],
                                         start=(t == 0), stop=(t == 1))
                    nc.vector.tensor_copy(ost[:, oc * 512:(oc + 1) * 512],
                                          oa[:])
                nc.sync.dma_start(out[mt * 128:(mt + 1) * 128, :], ost[:])

        # ---- emission ------------------------------------------------------
        with tc.tile_pool(name="psw", bufs=1, space="PSUM") as ps_w:
            st0 = ps_w.tile([16, 512], F32, tag="st", name="st0")
            for ch in range(NCH):
                for m in range(3):
                    proj_unit(ps_w, 0, m, ch, st0)
            lnexp(0, st0)

            # fin(p0) interleaved with P(p1)
            st1 = ps_w.tile([16, 512], F32, tag="st", name="st1")
            p1_units = [(m, ch) for ch in range(NCH) for m in range(3)]
            f0 = fin_chunks(0)
            fi = 0
            for i, (m, ch) in enumerate(p1_units):
                proj_unit(ps_w, 1, m, ch, st1)
                take = ((i + 1) * len(f0)) // len(p1_units) - fi
                for _ in range(take):
                    f0[fi]()
                    fi += 1
            lnexp(1, st1)

            # A(pair0) interleaved with fin(p1)
            f1 = fin_chunks(1)
            a0 = [(hh, ib) for hh in (0, 1) for ib in range(NIB)]
            fi = 0
            for i, (hh, ib) in enumerate(a0):
                take = ((i + 1) * len(f1)) // len(a0) - fi
                for _ in range(take):
                    f1[fi]()
                    fi += 1
                attn_unit(0, hh, ib)

        # pair1 attention + division + out-projection, per i-block
        with tc.tile_pool(name="pso", bufs=2, space="PSUM") as ps_o:
            for ib in range(NIB):
                attn_unit(1, 0, ib)
                attn_unit(1, 1, ib)
                ydiv_unit(ps_o, 0, ib)
                ydiv_unit(ps_o, 1, ib)
                o_unit(ps_o, ib)

        if debug_dump:
            for p in (0, 1):
                nc.sync.dma_start(dbg["d_qsb"][p], qsbF[p][0][:])
                nc.sync.dma_start(dbg["d_kT"][p], kT_sb[p][:])
                nc.sync.dma_start(dbg["d_yP"][p], yP[p][:])
                nc.sync.dma_start(dbg["d_sums"][p], sums_sb[p][:])
            for h in range(HL):
                nc.sync.dma_start(dbg["d_qTz"][h], qTz[h][:])
            nc.sync.dma_start(dbg["d_v"], v_sb[:])

    nc.compile()
    return nc


def _host_inputs(x, w_attn, w_proj):
    """Build the 8 per-core input maps (bf16 device tensors)."""
    bf = ml_dtypes.bfloat16
    inv_freq = 1.0 / (10000.0 ** (np.arange(0, D_HEAD, 2, dtype=np.float32)
                                  / D_HEAD))
    t = np.arange(T, dtype=np.float32)
    freqs = np.einsum('i,j->ij', t, inv_freq)            # [T, 32]
    emb = np.concatenate([freqs, freqs], 1)              # [T, 64]
    cos64 = np.cos(emb).T                                # [64, T]
    sin64 = np.sin(emb).T
    sgn = np.where(np.arange(64) < 32, -1.0, 1.0)[:, None].astype(np.float32)
    sinNeg64 = sin64 * sgn
    cosT = np.concatenate([cos64, cos64], 0).astype(bf)  # [128, T]
    sinNegT = np.concatenate([sinNeg64, sinNeg64], 0).astype(bf)

    tri = (np.arange(128)[:, None] <= np.arange(128)[None, :]).astype(bf)

    selqk = np.zeros((128, 8, 16), np.float32)
    selbc = np.zeros((16, 8, 128), np.float32)
    for m in range(2):
        for ch in range(NCH):
            u = m * 4 + ch
            for p in range(128):
                c = 4 * ch + 2 * m + (1 if p >= 64 else 0)
                selqk[p, u, c] = 1.0
                selbc[c, u, p] = 1.0
    sel2 = np.zeros((2, 128), np.float32)
    sel2[0, 0:64] = 1.0
    sel2[1, 64:128] = 1.0

    zpadQ = np.zeros((64, T), np.float32)
    onescol = np.ones((128, HL * NTT), np.float32)

    wq = w_attn[:D_MODEL]
    wk = w_attn[D_MODEL:2 * D_MODEL]
    wv_full = w_attn[2 * D_MODEL:]

    in_maps = []
    for c in range(N_CORES):
        b, hg = c // 4, c % 4
        wA = np.zeros((2, D_MODEL, 384), np.float32)
        for hp in (0, 1):
            hs = slice((hg * 4 + 2 * hp) * D_HEAD,
                       (hg * 4 + 2 * hp + 2) * D_HEAD)     # 128 rows
            wA[hp, :, 0:128] = wq[hs].T
            wA[hp, :, 128:256] = wk[hs].T
            wA[hp, :, 256:384] = wv_full[hs].T
        wp_c = [w_proj[:, (hg * 4 + j) * D_HEAD:(hg * 4 + j + 1) * D_HEAD].T
                for j in range(HL)]
        wpP = np.stack([np.concatenate([wp_c[0], wp_c[1]], 0),
                        np.concatenate([wp_c[2], wp_c[3]], 0)])

        in_maps.append({
            "xT": np.ascontiguousarray(x[b].T).astype(bf),
            "wA": wA.astype(bf),
            "cosT": cosT, "sinNegT": sinNegT, "trimask": tri,
            "selqk": selqk.astype(bf), "selbc": selbc.astype(bf),
            "sel2": sel2.astype(bf),
            "zpadQ": zpadQ.astype(bf), "onescol": onescol.astype(bf),
            "wpP": wpP.astype(bf),
        })
    return in_maps


def kernel(x, w_attn, w_proj, _want_results=False):
    x = np.asarray(x, dtype=np.float32)
    w_attn = np.asarray(w_attn, dtype=np.float32)
    w_proj = np.asarray(w_proj, dtype=np.float32)

    if "nc" not in _cached:
        _cached["nc"] = _build()
    nc = _cached["nc"]

    in_maps = _host_inputs(x, w_attn, w_proj)
    res = run_bass_kernel_spmd(nc, in_maps, list(range(N_CORES)))

    full = np.zeros((B, T, D_MODEL), np.float32)
    for c in range(N_CORES):
        full[c // 4] += res.results[c]["out"]
    if _want_results:
        return full, res
    return full


# revision 26
# speedup vs baseline: 1.3000x; 1.2298x over previous
"""Causal self-attention (QK-RMSNorm + rotary, H=16, D=1024, B=2, T=2048) on 8 NeuronCores.

Sharding: core c handles batch b = c // 4 and heads 4*(c%4) .. 4*(c%4)+3,
processed as two head PAIRS. Each core computes the qkv projection for its
heads, causal attention, and a row-parallel slice of the output projection;
the host sums the 4 partial outputs per batch element.

v2 design (vs the fp32r baseline):
- All matmul operands in bf16 (x, weights, q, k, v, p, y): halves DMA/SBUF.
- Rotary via a cross-partition DMA shift (rot(q) = shift(q) * signed-sin)
  instead of a second full projection: saves ~57k PE cycles.
- RMS scale applied to raw q/k before rope (rope is norm-preserving and
  commutes with per-head scalars); stats batched into one ln+exp per pair.
- Attention: full-512-wide score matmuls, exp batched in [128,1024] pairs
  (amortizes ACT's per-instruction bubble), causal masking via a Pool-side
  tri multiply on the diagonal squares, y-matmuls restricted to [o:512].
- Softmax sums ride the v ones-column (partition 64/63 of yacc) as in the
  baseline; division is per i-block so the out-projection overlaps the tail
  of attention.
- Engine balance: PE does matmuls only; ACT does exps+stats; DVE does
  PSUM->SBUF copies and PSUM-operand muls; Pool (gpsimd) does SBUF-only
  muls/adds (rope combine, tri); DMA does the rotary shift and v transpose.
"""
import sys
sys.path.insert(0, '/opt/trn_rl_repo')

import numpy as np
import ml_dtypes
from contextlib import ExitStack

import concourse.bass as bass
import concourse.tile as tile
from concourse import bacc, mybir
from concourse.bass_utils import run_bass_kernel_spmd

F32 = mybir.dt.float32
BF = mybir.dt.bfloat16
AF = mybir.ActivationFunctionType

N_HEAD = 16
D_MODEL = 1024
D_HEAD = 64
B, T = 2, 2048
N_CORES = 8
HL = 4              # heads per core
KT = D_MODEL // 128  # 8 contraction tiles
NCH = T // 512      # 4 token chunks
NIB = T // 512      # 4 i-blocks
NTT = T // 128      # 16 j-tiles
SCALE = D_HEAD ** -0.5

_cached = {}


def _build(debug_dump=False):
    nc = bacc.Bacc("TRN2", target_bir_lowering=False, debug=False,
                   num_devices=N_CORES)

    # ---- DRAM I/O ----------------------------------------------------------
    xT = nc.dram_tensor("xT", [D_MODEL, T], BF, kind="ExternalInput").ap()
    wA = nc.dram_tensor("wA", [2, D_MODEL, 384], BF, kind="ExternalInput").ap()
    cosT = nc.dram_tensor("cosT", [128, T], BF, kind="ExternalInput").ap()
    sinNegT = nc.dram_tensor("sinNegT", [128, T], BF,
                             kind="ExternalInput").ap()
    trimask = nc.dram_tensor("trimask", [128, 128], BF,
                             kind="ExternalInput").ap()
    selqk = nc.dram_tensor("selqk", [128, 8, 16], BF,
                           kind="ExternalInput").ap()
    selbc = nc.dram_tensor("selbc", [16, 8, 128], BF,
                           kind="ExternalInput").ap()
    sel2 = nc.dram_tensor("sel2", [2, 128], BF, kind="ExternalInput").ap()
    zpadQ = nc.dram_tensor("zpadQ", [64, T], BF, kind="ExternalInput").ap()
    onescol = nc.dram_tensor("onescol", [128, HL * NTT], BF,
                             kind="ExternalInput").ap()
    wpP = nc.dram_tensor("wpP", [2, 128, 1024], BF, kind="ExternalInput").ap()
    out = nc.dram_tensor("out", [T, D_MODEL], F32, kind="ExternalOutput").ap()
    if debug_dump:
        dbg = {
            "d_qsb": nc.dram_tensor("d_qsb", [2, 128, T], BF,
                                    kind="ExternalOutput").ap(),
            "d_qTz": nc.dram_tensor("d_qTz", [HL, 128, T], BF,
                                    kind="ExternalOutput").ap(),
            "d_kT": nc.dram_tensor("d_kT", [2, 128, T], BF,
                                   kind="ExternalOutput").ap(),
            "d_v": nc.dram_tensor("d_v", [128, 5200], BF,
                                  kind="ExternalOutput").ap(),
            "d_sums": nc.dram_tensor("d_sums", [2, 2, NIB, 512], F32,
                                     kind="ExternalOutput").ap(),
            "d_yP": nc.dram_tensor("d_yP", [2, 128, T], BF,
                                   kind="ExternalOutput").ap(),
        }

    with tile.TileContext(nc) as tc, ExitStack() as ctx:
        ctx.enter_context(nc.allow_low_precision(
            reason="bf16 matmuls/intermediates; tolerance is 2e-2"))

        cpool = ctx.enter_context(tc.tile_pool(name="consts", bufs=1))
        work = ctx.enter_context(tc.tile_pool(name="work", bufs=2))
        ps_s = ctx.enter_context(tc.tile_pool(name="pss", bufs=2,
                                              space="PSUM"))
        ps_y = ctx.enter_context(tc.tile_pool(name="psy", bufs=1,
                                              space="PSUM"))

        # ---- persistent SBUF -----------------------------------------------
        x_sb = cpool.tile([128, KT, T], BF)
        wA_sb = [cpool.tile([128, KT, 384], BF, name=f"wA{p}") for p in (0, 1)]
        cos_sb = cpool.tile([128, T], BF)
        sinNeg_sb = cpool.tile([128, T], BF)
        tri_sb = cpool.tile([128, 128], BF)
        selqk_sb = cpool.tile([128, 8, 16], BF)
        selbc_sb = cpool.tile([16, 8, 128], BF)
        sel2_sb = cpool.tile([2, 128], BF)
        wpP_sb = [cpool.tile([128, 1024], BF, name=f"wpP{p}") for p in (0, 1)]
        qTz = [cpool.tile([128, T], BF, name=f"qTz{h}") for h in range(HL)]
        kT_sb = [cpool.tile([128, T], BF, name=f"kT{p}") for p in (0, 1)]
        # padded past HL*NTT*65+64 so the [p, 2, 1040] transpose-dst view of
        # the last head pair stays in bounds
        v_sb = cpool.tile([128, 5200], BF)
        v3 = v_sb[:, 0:HL * NTT * 65].rearrange("p (g o) -> p g o", o=65)
        yP = [cpool.tile([128, T], BF, name=f"yP{p}") for p in (0, 1)]
        qsbF = [[cpool.tile([128, T], BF, name=f"qsb{p}{m}") for m in (0, 1)]
                for p in (0, 1)]
        qshF = [[cpool.tile([128, T], BF, name=f"qsh{p}{m}") for m in (0, 1)]
                for p in (0, 1)]
        rinv_sb = [cpool.tile([16, 512], BF, name=f"rinv{p}") for p in (0, 1)]
        sums_sb = [cpool.tile([2, NIB, 512], F32, name=f"sums{p}")
                   for p in (0, 1)]
        rinvy_sb = [cpool.tile([2, NIB, 512], BF, name=f"rinvy{p}")
                    for p in (0, 1)]

        # ---- preamble DMAs -------------------------------------------------
        nc.sync.dma_start(wA_sb[0][:],
                          wA[0].rearrange("(k p) c -> p k c", p=128))
        for ch in range(NCH):
            nc.sync.dma_start(x_sb[:, :, ch * 512:(ch + 1) * 512],
                              xT.rearrange("(k p) t -> p k t", p=128)
                              [:, :, ch * 512:(ch + 1) * 512])
        nc.sync.dma_start(selqk_sb[:], selqk[:])
        nc.sync.dma_start(wA_sb[1][:],
                          wA[1].rearrange("(k p) c -> p k c", p=128))
        nc.sync.dma_start(cos_sb[:], cosT[:])
        nc.sync.dma_start(sinNeg_sb[:], sinNegT[:])
        nc.sync.dma_start(tri_sb[:], trimask[:])
        nc.sync.dma_start(selbc_sb[:], selbc[:])
        nc.sync.dma_start(sel2_sb[:], sel2[:])
        for h in range(HL):
            half = slice(64, 128) if h % 2 == 0 else slice(0, 64)
            nc.sync.dma_start(qTz[h][half, :], zpadQ[:])
        nc.sync.dma_start(v3[:, :, 64:65], onescol.unsqueeze(2))
        for p in (0, 1):
            nc.sync.dma_start(wpP_sb[p][:], wpP[p])

        ySG_store = {}

        # ---- unit emitters -------------------------------------------------
        def proj_unit(ps_w, hp, m, ch, st_tile):
            cs = slice(ch * 512, (ch + 1) * 512)
            acc = ps_w.tile([128, 512], F32, tag="pa", bufs=2, name="acc")
            for k in range(KT):
                nc.tensor.matmul(acc[:],
                                 wA_sb[hp][:, k, m * 128:(m + 1) * 128],
                                 x_sb[:, k, cs], start=(k == 0),
                                 stop=(k == KT - 1))
            if m < 2:
                dst = qsbF[hp][m][:, cs]
                nc.vector.tensor_copy(dst, acc[:])
                if ch == NCH - 1:
                    # all 4 chunks landed: batched square + 4 stats matmuls
                    sqF = work.tile([128, T], BF, tag="sqF", bufs=1,
                                    name="sqF")
                    nc.gpsimd.tensor_mul(sqF[:], qsbF[hp][m][:],
                                         qsbF[hp][m][:])
                    for c2 in range(NCH):
                        idx = m * 4 + c2
                        nc.tensor.matmul(
                            st_tile[:], selqk_sb[:, idx, :],
                            sqF[:, c2 * 512:(c2 + 1) * 512],
                            start=(idx == 0), stop=(idx == 7))
            else:
                vdst = work.tile([128, 512], BF, tag="vsb", bufs=3,
                                 name="vdst")
                nc.vector.tensor_copy(vdst[:], acc[:])
                for s4 in range(4):
                    jt = ch * 4 + s4
                    gA = (2 * hp) * NTT + jt
                    vstg = work.tile([128, 128], BF, tag="vstg", bufs=3,
                                     name="vstg")
                    nc.sync.dma_start_transpose(
                        vstg[:], vdst[:, s4 * 128:(s4 + 1) * 128])
                    vv = v_sb[:, gA * 65:gA * 65 + 2080] \
                        .rearrange("p (h x) -> p h x", h=2)[:, :, 0:64]
                    nc.gpsimd.tensor_copy(
                        vv, vstg[:].rearrange("p (h x) -> p h x", h=2))

        def lnexp(hp, st_tile):
            lnt = work.tile([16, 512], F32, tag="lnt", bufs=1, name="lnt")
            nc.scalar.activation(lnt[:], st_tile[:], AF.Ln, scale=1.0 / 64.0)
            nc.scalar.activation(rinv_sb[hp][:], lnt[:], AF.Exp, scale=-0.5)

        def fin_chunks(hp):
            """Chunk closures: rms-scale in place, rotary shift + combine.

            Full-T granularity: t1 overwrites qsbF, t2 overwrites qshF
            (both dead afterwards), so no scratch tiles are needed."""
            chunks = []
            for m in (0, 1):
                for ch in range(NCH):
                    def bc_scale(m=m, ch=ch):
                        cs = slice(ch * 512, (ch + 1) * 512)
                        bc = ps_s.tile([128, 512], F32, tag="s", name="bc")
                        nc.tensor.matmul(bc[:], selbc_sb[:, m * 4 + ch, :],
                                         rinv_sb[hp][:], start=True,
                                         stop=True)
                        nc.vector.tensor_mul(qsbF[hp][m][:, cs],
                                             qsbF[hp][m][:, cs], bc[:])
                    chunks.append(bc_scale)

                def shifts(m=m):
                    for blk in range(4):
                        d0 = blk * 32
                        s0 = (blk ^ 1) * 32
                        nc.sync.dma_start(qshF[hp][m][d0:d0 + 32, :],
                                          qsbF[hp][m][s0:s0 + 32, :])
                chunks.append(shifts)

            def rope_mul(m, which):
                if which == 0:
                    nc.gpsimd.tensor_mul(qsbF[hp][m][:], qsbF[hp][m][:],
                                         cos_sb[:])
                else:
                    nc.gpsimd.tensor_mul(qshF[hp][m][:], qshF[hp][m][:],
                                         sinNeg_sb[:])

            def rope_add(m):
                if m == 0:
                    nc.vector.tensor_add(qTz[2 * hp][0:64, :],
                                         qsbF[hp][0][0:64, :],
                                         qshF[hp][0][0:64, :])
                    nc.vector.tensor_add(qTz[2 * hp + 1][64:128, :],
                                         qsbF[hp][0][64:128, :],
                                         qshF[hp][0][64:128, :])
                else:
                    nc.vector.tensor_add(kT_sb[hp][:], qsbF[hp][1][:],
                                         qshF[hp][1][:])

            chunks.append(lambda: rope_mul(0, 1))
            chunks.append(lambda: rope_mul(1, 1))
            chunks.append(lambda: rope_mul(0, 0))
            chunks.append(lambda: rope_add(0))
            chunks.append(lambda: rope_mul(1, 0))
            chunks.append(lambda: rope_add(1))
            return chunks

        def attn_unit(hp, hh, ib):
            h_l = 2 * hp + hh
            njt = 4 * (ib + 1)
            ibs = ib * 512
            yacc = ps_y.tile([128, 512], F32, tag="y", name="yacc")

            def ymms(pr, pt):
                for half in (0, 1):
                    jt = 2 * pr + half
                    o = max(0, jt * 128 - ibs)
                    g = h_l * NTT + jt
                    if hh == 0:
                        vau = v_sb[:, g * 65:g * 65 + 128]
                    else:
                        vau = v_sb[:, g * 65 - 64:g * 65 + 64]
                    nc.tensor.matmul(yacc[:, o:512], vau,
                                     pt[:, half * 512 + o:half * 512 + 512],
                                     start=(jt == 0), stop=(jt == njt - 1))

            prev = None
            for pr in range(njt // 2):
                sp = ps_s.tile([128, 1024], F32, tag="s", name="sp")
                for half in (0, 1):
                    jt = 2 * pr + half
                    nc.tensor.matmul(sp[:, half * 512:(half + 1) * 512],
                                     kT_sb[hp][:, jt * 128:(jt + 1) * 128],
                                     qTz[h_l][:, ibs:ibs + 512],
                                     start=True, stop=True)
                pt = work.tile([128, 1024], BF, tag="p", bufs=3, name="pt")
                nc.scalar.activation(pt[:], sp[:], AF.Exp, scale=SCALE)
                for half in (0, 1):
                    jt = 2 * pr + half
                    o = jt * 128 - ibs
                    if o >= 0:
                        lo = half * 512 + o
                        nc.vector.tensor_mul(pt[:, lo:lo + 128],
                                             pt[:, lo:lo + 128], tri_sb[:])
                if prev is not None:
                    ymms(*prev)
                prev = (pr, pt)
            ymms(*prev)

            ySG = work.tile([128, 512], F32, tag="ysg", bufs=10, name="ySG")
            nc.vector.tensor_copy(ySG[:], yacc[:])
            srow = 64 if hh == 0 else 63
            nc.sync.dma_start(sums_sb[hp][hh:hh + 1, ib, :],
                              ySG[srow:srow + 1, :])
            ySG_store[(hp, hh, ib)] = ySG

        def ydiv_unit(ps_o, hp, ib):
            ibs = ib * 512
            # 1/x as exp(-ln(x)): ACT is far cheaper than DVE reciprocal
            lns = work.tile([2, 512], F32, tag="lns", bufs=1, name="lns")
            nc.scalar.activation(lns[:], sums_sb[hp][:, ib, :], AF.Ln)
            nc.scalar.activation(rinvy_sb[hp][:, ib, :], lns[:], AF.Exp,
                                 scale=-1.0)
            bc2 = ps_o.tile([128, 512], F32, tag="bc2", bufs=1, name="bc2")
            nc.tensor.matmul(bc2[:], sel2_sb[:], rinvy_sb[hp][:, ib, :],
                             start=True, stop=True)
            e = ySG_store[(hp, 0, ib)]
            o_ = ySG_store[(hp, 1, ib)]
            nc.vector.tensor_mul(yP[hp][0:64, ibs:ibs + 512], e[0:64, :],
                                 bc2[0:64, :])
            nc.vector.tensor_mul(yP[hp][64:128, ibs:ibs + 512],
                                 o_[64:128, :], bc2[64:128, :])

        def o_unit(ps_o, ib):
            for mt in range(4 * ib, 4 * ib + 4):
                ost = work.tile([128, 1024], F32, tag="ost", name="ost")
                oa = ps_o.tile([128, 1024], F32, tag="oa", bufs=1, name="oa")
                for oc in (0, 1):
                    for t in (0, 1):
                        nc.tensor.matmul(oa[:, oc * 512:(oc + 1) * 512],
                                         yP[t][:, mt * 128:(mt + 1) * 128],
                                         wpP_sb[t][:, oc * 512:(oc + 1) * 512],
                                         start=(t == 0), stop=(t == 1))
                nc.vector.tensor_copy(ost[:], oa[:])
                nc.sync.dma_start(out[mt * 128:(mt + 1) * 128, :], ost[:])

        # ---- emission ------------------------------------------------------
        with tc.tile_pool(name="psw", bufs=1, space="PSUM") as ps_w:
            st0 = ps_w.tile([16, 512], F32, tag="st", name="st0")
            for m in range(3):
                for ch in range(NCH):
                    proj_unit(ps_w, 0, m, ch, st0)
            lnexp(0, st0)

            # fin(p0) interleaved with P(p1)
            st1 = ps_w.tile([16, 512], F32, tag="st", name="st1")
            p1_units = [(m, ch) for m in range(3) for ch in range(NCH)]
            f0 = fin_chunks(0)
            fi = 0
            for i, (m, ch) in enumerate(p1_units):
                proj_unit(ps_w, 1, m, ch, st1)
                take = ((i + 1) * len(f0)) // len(p1_units) - fi
                for _ in range(take):
                    f0[fi]()
                    fi += 1
            lnexp(1, st1)

            # A(pair0) interleaved with fin(p1)
            f1 = fin_chunks(1)
            a0 = [(hh, ib) for hh in (0, 1) for ib in range(NIB)]
            fi = 0
            for i, (hh, ib) in enumerate(a0):
                take = ((i + 1) * len(f1)) // len(a0) - fi
                for _ in range(take):
                    f1[fi]()
                    fi += 1
                attn_unit(0, hh, ib)

        # pair1 attention + division + out-projection, per i-block
        with tc.tile_pool(name="pso", bufs=2, space="PSUM") as ps_o:
            for ib in range(NIB):
                attn_unit(1, 0, ib)
                attn_unit(1, 1, ib)
                ydiv_unit(ps_o, 0, ib)
                ydiv_unit(ps_o, 1, ib)
                o_unit(ps_o, ib)

        if debug_dump:
            for p in (0, 1):
                nc.sync.dma_start(dbg["d_qsb"][p], qsbF[p][0][:])
                nc.sync.dma_start(dbg["d_kT"][p], kT_sb[p][:])
                nc.sync.dma_start(dbg["d_yP"][p], yP[p][:])
                nc.sync.dma_start(dbg["d_sums"][p], sums_sb[p][:])
            for h in range(HL):
                nc.sync.dma_start(dbg["d_qTz"][h], qTz[h][:])
            nc.sync.dma_start(dbg["d_v"], v_sb[:])

    nc.compile()
    return nc


def _host_inputs(x, w_attn, w_proj):
    """Build the 8 per-core input maps (bf16 device tensors)."""
    bf = ml_dtypes.bfloat16
    inv_freq = 1.0 / (10000.0 ** (np.arange(0, D_HEAD, 2, dtype=np.float32)
                                  / D_HEAD))
    t = np.arange(T, dtype=np.float32)
    freqs = np.einsum('i,j->ij', t, inv_freq)            # [T, 32]
    emb = np.concatenate([freqs, freqs], 1)              # [T, 64]
    cos64 = np.cos(emb).T                                # [64, T]
    sin64 = np.sin(emb).T
    sgn = np.where(np.arange(64) < 32, -1.0, 1.0)[:, None].astype(np.float32)
    sinNeg64 = sin64 * sgn
    cosT = np.concatenate([cos64, cos64], 0).astype(bf)  # [128, T]
    sinNegT = np.concatenate([sinNeg64, sinNeg64], 0).astype(bf)

    tri = (np.arange(128)[:, None] <= np.arange(128)[None, :]).astype(bf)

    selqk = np.zeros((128, 8, 16), np.float32)
    selbc = np.zeros((16, 8, 128), np.float32)
    for m in range(2):
        for ch in range(NCH):
            u = m * 4 + ch
            for p in range(128):
                c = 4 * ch + 2 * m + (1 if p >= 64 else 0)
                selqk[p, u, c] = 1.0
                selbc[c, u, p] = 1.0
    sel2 = np.zeros((2, 128), np.float32)
    sel2[0, 0:64] = 1.0
    sel2[1, 64:128] = 1.0

    zpadQ = np.zeros((64, T), np.float32)
    onescol = np.ones((128, HL * NTT), np.float32)

    wq = w_attn[:D_MODEL]
    wk = w_attn[D_MODEL:2 * D_MODEL]
    wv_full = w_attn[2 * D_MODEL:]

    in_maps = []
    for c in range(N_CORES):
        b, hg = c // 4, c % 4
        wA = np.zeros((2, D_MODEL, 384), np.float32)
        for hp in (0, 1):
            hs = slice((hg * 4 + 2 * hp) * D_HEAD,
                       (hg * 4 + 2 * hp + 2) * D_HEAD)     # 128 rows
            wA[hp, :, 0:128] = wq[hs].T
            wA[hp, :, 128:256] = wk[hs].T
            wA[hp, :, 256:384] = wv_full[hs].T
        wp_c = [w_proj[:, (hg * 4 + j) * D_HEAD:(hg * 4 + j + 1) * D_HEAD].T
                for j in range(HL)]
        wpP = np.stack([np.concatenate([wp_c[0], wp_c[1]], 0),
                        np.concatenate([wp_c[2], wp_c[3]], 0)])

        in_maps.append({
            "xT": np.ascontiguousarray(x[b].T).astype(bf),
            "wA": wA.astype(bf),
            "cosT": cosT, "sinNegT": sinNegT, "trimask": tri,
            "selqk": selqk.astype(bf), "selbc": selbc.astype(bf),
            "sel2": sel2.astype(bf),
            "zpadQ": zpadQ.astype(bf), "onescol": onescol.astype(bf),
            "wpP": wpP.astype(bf),
        })
    return in_maps


def kernel(x, w_attn, w_proj, _want_results=False):
    x = np.asarray(x, dtype=np.float32)
    w_attn = np.asarray(w_attn, dtype=np.float32)
    w_proj = np.asarray(w_proj, dtype=np.float32)

    if "nc" not in _cached:
        _cached["nc"] = _build()
    nc = _cached["nc"]

    in_maps = _host_inputs(x, w_attn, w_proj)
    res = run_bass_kernel_spmd(nc, in_maps, list(range(N_CORES)))

    full = np.zeros((B, T, D_MODEL), np.float32)
    for c in range(N_CORES):
        full[c // 4] += res.results[c]["out"]
    if _want_results:
        return full, res
    return full


# revision 41
# speedup vs baseline: 1.3686x; 1.0528x over previous
"""Causal self-attention (QK-RMSNorm + rotary, H=16, D=1024, B=2, T=2048) on 8 NeuronCores.

Sharding: core c handles batch b = c // 4 and heads 4*(c%4) .. 4*(c%4)+3,
processed as two head PAIRS. Each core computes the qkv projection for its
heads, causal attention, and a row-parallel slice of the output projection;
the host sums the 4 partial outputs per batch element.

v2 design (vs the fp32r baseline):
- All matmul operands in bf16 (x, weights, q, k, v, p, y): halves DMA/SBUF.
- Rotary via a cross-partition DMA shift (rot(q) = shift(q) * signed-sin)
  instead of a second full projection: saves ~57k PE cycles.
- RMS scale applied to raw q/k before rope (rope is norm-preserving and
  commutes with per-head scalars); stats batched into one ln+exp per pair.
- Attention: full-512-wide score matmuls, exp batched in [128,1024] pairs
  (amortizes ACT's per-instruction bubble), causal masking via a Pool-side
  tri multiply on the diagonal squares, y-matmuls restricted to [o:512].
- Softmax sums ride the v ones-column (partition 64/63 of yacc) as in the
  baseline; division is per i-block so the out-projection overlaps the tail
  of attention.
- Engine balance: PE does matmuls only; ACT does exps+stats; DVE does
  PSUM->SBUF copies and PSUM-operand muls; Pool (gpsimd) does SBUF-only
  muls/adds (rope combine, tri); DMA does the rotary shift and v transpose.
"""
import sys
sys.path.insert(0, '/opt/trn_rl_repo')

import numpy as np
import ml_dtypes
from contextlib import ExitStack

import concourse.bass as bass
import concourse.tile as tile
from concourse import bacc, mybir
from concourse.bass_utils import run_bass_kernel_spmd

F32 = mybir.dt.float32
BF = mybir.dt.bfloat16
AF = mybir.ActivationFunctionType

N_HEAD = 16
D_MODEL = 1024
D_HEAD = 64
B, T = 2, 2048
N_CORES = 8
HL = 4              # heads per core
KT = D_MODEL // 128  # 8 contraction tiles
NCH = T // 512      # 4 token chunks
NIB = T // 512      # 4 i-blocks
NTT = T // 128      # 16 j-tiles
SCALE = D_HEAD ** -0.5

_cached = {}


def _build(debug_dump=False):
    nc = bacc.Bacc("TRN2", target_bir_lowering=False, debug=False,
                   num_devices=N_CORES)

    # ---- DRAM I/O ----------------------------------------------------------
    xT = nc.dram_tensor("xT", [D_MODEL, T], BF, kind="ExternalInput").ap()
    wA = nc.dram_tensor("wA", [2, D_MODEL, 384], BF, kind="ExternalInput").ap()
    cosT = nc.dram_tensor("cosT", [128, T], BF, kind="ExternalInput").ap()
    sinNegT = nc.dram_tensor("sinNegT", [128, T], BF,
                             kind="ExternalInput").ap()
    trimask = nc.dram_tensor("trimask", [128, 128], BF,
                             kind="ExternalInput").ap()
    selqk = nc.dram_tensor("selqk", [128, 8, 16], BF,
                           kind="ExternalInput").ap()
    selbc = nc.dram_tensor("selbc", [16, 8, 128], BF,
                           kind="ExternalInput").ap()
    sel4 = nc.dram_tensor("sel4", [4, 2, 128], BF, kind="ExternalInput").ap()
    zpadQ = nc.dram_tensor("zpadQ", [64, T], BF, kind="ExternalInput").ap()
    onescol = nc.dram_tensor("onescol", [128, HL * NTT], BF,
                             kind="ExternalInput").ap()
    wpP = nc.dram_tensor("wpP", [2, 128, 1024], BF, kind="ExternalInput").ap()
    out = nc.dram_tensor("out", [T, D_MODEL], F32, kind="ExternalOutput").ap()
    if debug_dump:
        dbg = {
            "d_qsb": nc.dram_tensor("d_qsb", [2, 128, T], BF,
                                    kind="ExternalOutput").ap(),
            "d_qTz": nc.dram_tensor("d_qTz", [HL, 128, T], BF,
                                    kind="ExternalOutput").ap(),
            "d_kT": nc.dram_tensor("d_kT", [2, 128, T], BF,
                                   kind="ExternalOutput").ap(),
            "d_v": nc.dram_tensor("d_v", [128, 5200], BF,
                                  kind="ExternalOutput").ap(),
            "d_sums": nc.dram_tensor("d_sums", [4, NIB, 512], F32,
                                     kind="ExternalOutput").ap(),
            "d_yP": nc.dram_tensor("d_yP", [2, 128, T], BF,
                                   kind="ExternalOutput").ap(),
        }

    with tile.TileContext(nc) as tc, ExitStack() as ctx:
        ctx.enter_context(nc.allow_low_precision(
            reason="bf16 matmuls/intermediates; tolerance is 2e-2"))

        cpool = ctx.enter_context(tc.tile_pool(name="consts", bufs=1))
        work = ctx.enter_context(tc.tile_pool(name="work", bufs=2))
        ps_s = ctx.enter_context(tc.tile_pool(name="pss", bufs=2,
                                              space="PSUM"))
        ps_y = ctx.enter_context(tc.tile_pool(name="psy", bufs=1,
                                              space="PSUM"))

        # ---- persistent SBUF -----------------------------------------------
        x_sb = cpool.tile([128, KT, T], BF)
        wA_sb = [cpool.tile([128, KT, 384], BF, name=f"wA{p}") for p in (0, 1)]
        cos_sb = cpool.tile([128, T], BF)
        sinNeg_sb = cpool.tile([128, T], BF)
        tri_sb = cpool.tile([128, 128], BF)
        selqk_sb = cpool.tile([128, 8, 16], BF)
        selbc_sb = cpool.tile([16, 8, 128], BF)
        sel4_sb = cpool.tile([4, 2, 128], BF)
        wpP_sb = [cpool.tile([128, 1024], BF, name=f"wpP{p}") for p in (0, 1)]
        qTz = [cpool.tile([128, T], BF, name=f"qTz{h}") for h in range(HL)]
        kT_sb = [cpool.tile([128, T], BF, name=f"kT{p}") for p in (0, 1)]
        # padded past HL*NTT*65+64 so the [p, 2, 1040] transpose-dst view of
        # the last head pair stays in bounds
        v_sb = cpool.tile([128, 5200], BF)
        v3 = v_sb[:, 0:HL * NTT * 65].rearrange("p (g o) -> p g o", o=65)
        yP = [cpool.tile([128, T], BF, name=f"yP{p}") for p in (0, 1)]
        qsbF = [[cpool.tile([128, T], BF, name=f"qsb{p}{m}") for m in (0, 1)]
                for p in (0, 1)]
        qshF = [[cpool.tile([128, T], BF, name=f"qsh{p}{m}") for m in (0, 1)]
                for p in (0, 1)]
        rinv_sb = [cpool.tile([16, 512], BF, name=f"rinv{p}") for p in (0, 1)]
        sums_sb = cpool.tile([4, NIB, 512], F32)
        rinvy_sb = cpool.tile([4, NIB, 512], BF)

        # ---- preamble DMAs -------------------------------------------------
        nc.sync.dma_start(wA_sb[0][:],
                          wA[0].rearrange("(k p) c -> p k c", p=128))
        for ch in range(NCH):
            nc.sync.dma_start(x_sb[:, :, ch * 512:(ch + 1) * 512],
                              xT.rearrange("(k p) t -> p k t", p=128)
                              [:, :, ch * 512:(ch + 1) * 512])
        nc.sync.dma_start(selqk_sb[:], selqk[:])
        nc.sync.dma_start(wA_sb[1][:],
                          wA[1].rearrange("(k p) c -> p k c", p=128))
        nc.sync.dma_start(cos_sb[:], cosT[:])
        nc.sync.dma_start(sinNeg_sb[:], sinNegT[:])
        nc.sync.dma_start(tri_sb[:], trimask[:])
        nc.sync.dma_start(selbc_sb[:], selbc[:])
        nc.sync.dma_start(sel4_sb[:], sel4[:])
        for h in range(HL):
            half = slice(64, 128) if h % 2 == 0 else slice(0, 64)
            nc.sync.dma_start(qTz[h][half, :], zpadQ[:])
        nc.sync.dma_start(v3[:, :, 64:65], onescol.unsqueeze(2))
        for p in (0, 1):
            nc.sync.dma_start(wpP_sb[p][:], wpP[p])

        ySG_store = {}

        # ---- unit emitters -------------------------------------------------
        def proj_unit(ps_w, hp, m, ch, st_tile):
            cs = slice(ch * 512, (ch + 1) * 512)
            acc = ps_w.tile([128, 512], F32, tag="pa", bufs=2, name="acc")
            for k in range(KT):
                nc.tensor.matmul(acc[:],
                                 wA_sb[hp][:, k, m * 128:(m + 1) * 128],
                                 x_sb[:, k, cs], start=(k == 0),
                                 stop=(k == KT - 1))
            if m < 2:
                dst = qsbF[hp][m][:, cs]
                nc.vector.tensor_copy(dst, acc[:])
                if ch == NCH - 1:
                    # all 4 chunks landed: batched square + 4 stats matmuls
                    sqF = work.tile([128, T], BF, tag="sqF", bufs=1,
                                    name="sqF")
                    nc.vector.tensor_mul(sqF[:], qsbF[hp][m][:],
                                         qsbF[hp][m][:])
                    for c2 in range(NCH):
                        idx = m * 4 + c2
                        nc.tensor.matmul(
                            st_tile[:], selqk_sb[:, idx, :],
                            sqF[:, c2 * 512:(c2 + 1) * 512],
                            start=(idx == 0), stop=(idx == 7))
            else:
                vdst = work.tile([128, 512], BF, tag="vsb", bufs=3,
                                 name="vdst")
                nc.vector.tensor_copy(vdst[:], acc[:])
                for s4 in range(4):
                    jt = ch * 4 + s4
                    gA = (2 * hp) * NTT + jt
                    vstg = work.tile([128, 128], BF, tag="vstg", bufs=3,
                                     name="vstg")
                    nc.scalar.dma_start_transpose(
                        vstg[:], vdst[:, s4 * 128:(s4 + 1) * 128])
                    vv = v_sb[:, gA * 65:gA * 65 + 2080] \
                        .rearrange("p (h x) -> p h x", h=2)[:, :, 0:64]
                    nc.gpsimd.tensor_copy(
                        vv, vstg[:].rearrange("p (h x) -> p h x", h=2))

        def lnexp(hp, st_tile):
            lnt = work.tile([16, 512], F32, tag="lnt", bufs=1, name="lnt")
            nc.scalar.activation(lnt[:], st_tile[:], AF.Ln, scale=1.0 / 64.0)
            nc.scalar.activation(rinv_sb[hp][:], lnt[:], AF.Exp, scale=-0.5)

        def fin_chunks(hp):
            """Chunk closures: rms-scale in place, rotary shift + combine.

            Full-T granularity: t1 overwrites qsbF, t2 overwrites qshF
            (both dead afterwards), so no scratch tiles are needed."""
            chunks = []
            for m in (0, 1):
                for ch in range(NCH):
                    def bc_scale(m=m, ch=ch):
                        cs = slice(ch * 512, (ch + 1) * 512)
                        bc = ps_s.tile([128, 512], F32, tag="s", name="bc")
                        nc.tensor.matmul(bc[:], selbc_sb[:, m * 4 + ch, :],
                                         rinv_sb[hp][:], start=True,
                                         stop=True)
                        nc.vector.tensor_mul(qsbF[hp][m][:, cs],
                                             qsbF[hp][m][:, cs], bc[:])
                    chunks.append(bc_scale)

                def shifts(m=m):
                    for blk in range(4):
                        d0 = blk * 32
                        s0 = (blk ^ 1) * 32
                        nc.scalar.dma_start(qshF[hp][m][d0:d0 + 32, :],
                                            qsbF[hp][m][s0:s0 + 32, :])
                chunks.append(shifts)

            def rope_mul(m, which):
                if which == 0:
                    nc.vector.tensor_mul(qsbF[hp][m][:], qsbF[hp][m][:],
                                         cos_sb[:])
                else:
                    nc.vector.tensor_mul(qshF[hp][m][:], qshF[hp][m][:],
                                         sinNeg_sb[:])

            def rope_add(m):
                if m == 0:
                    nc.vector.tensor_add(qTz[2 * hp][0:64, :],
                                         qsbF[hp][0][0:64, :],
                                         qshF[hp][0][0:64, :])
                    nc.vector.tensor_add(qTz[2 * hp + 1][64:128, :],
                                         qsbF[hp][0][64:128, :],
                                         qshF[hp][0][64:128, :])
                else:
                    nc.vector.tensor_add(kT_sb[hp][:], qsbF[hp][1][:],
                                         qshF[hp][1][:])

            chunks.append(lambda: rope_mul(0, 1))
            chunks.append(lambda: rope_mul(1, 1))
            chunks.append(lambda: rope_mul(0, 0))
            chunks.append(lambda: rope_add(0))
            chunks.append(lambda: rope_mul(1, 0))
            chunks.append(lambda: rope_add(1))
            return chunks

        def attn_unit(hp, hh, ib):
            h_l = 2 * hp + hh
            njt = 4 * (ib + 1)
            ibs = ib * 512
            yacc = ps_y.tile([128, 512], F32, tag="y", name="yacc")

            def ymms(pr, pt):
                for half in (0, 1):
                    jt = 2 * pr + half
                    o = max(0, jt * 128 - ibs)
                    g = h_l * NTT + jt
                    if hh == 0:
                        vau = v_sb[:, g * 65:g * 65 + 128]
                    else:
                        vau = v_sb[:, g * 65 - 64:g * 65 + 64]
                    nc.tensor.matmul(yacc[:, o:512], vau,
                                     pt[:, half * 512 + o:half * 512 + 512],
                                     start=(jt == 0), stop=(jt == njt - 1))

            prev = None
            for pr in range(njt // 2):
                sp = ps_s.tile([128, 1024], F32, tag="s", name="sp")
                for half in (0, 1):
                    jt = 2 * pr + half
                    nc.tensor.matmul(sp[:, half * 512:(half + 1) * 512],
                                     kT_sb[hp][:, jt * 128:(jt + 1) * 128],
                                     qTz[h_l][:, ibs:ibs + 512],
                                     start=True, stop=True)
                pt = work.tile([128, 1024], BF, tag="p", bufs=3, name="pt")
                nc.scalar.activation(pt[:], sp[:], AF.Exp, scale=SCALE)
                for half in (0, 1):
                    jt = 2 * pr + half
                    o = jt * 128 - ibs
                    if o >= 0:
                        lo = half * 512 + o
                        nc.vector.tensor_mul(pt[:, lo:lo + 128],
                                             pt[:, lo:lo + 128], tri_sb[:])
                if prev is not None:
                    ymms(*prev)
                prev = (pr, pt)
            ymms(*prev)

            ySG = work.tile([128, 512], F32, tag="ysg", bufs=10, name="ySG")
            nc.vector.tensor_copy(ySG[:], yacc[:])
            srow = 64 if hh == 0 else 63
            nc.sync.dma_start(sums_sb[2 * hp + hh:2 * hp + hh + 1, ib, :],
                              ySG[srow:srow + 1, :])
            ySG_store[(hp, hh, ib)] = ySG

        def ydiv_unit(ps_o, hp, ib):
            ibs = ib * 512
            if hp == 0:
                # one reciprocal per i-block covers both pairs (cost is
                # free-size-dominated, so batching partitions is free)
                nc.vector.reciprocal(rinvy_sb[:, ib, :], sums_sb[:, ib, :])
            bc2 = ps_o.tile([128, 512], F32, tag="bc2", bufs=1, name="bc2")
            nc.tensor.matmul(bc2[:], sel4_sb[:, hp, :], rinvy_sb[:, ib, :],
                             start=True, stop=True)
            e = ySG_store[(hp, 0, ib)]
            o_ = ySG_store[(hp, 1, ib)]
            nc.vector.tensor_mul(yP[hp][0:64, ibs:ibs + 512], e[0:64, :],
                                 bc2[0:64, :])
            nc.vector.tensor_mul(yP[hp][64:128, ibs:ibs + 512],
                                 o_[64:128, :], bc2[64:128, :])

        def o_unit(ps_o, ib):
            for mt in range(4 * ib, 4 * ib + 4):
                ost = work.tile([128, 1024], F32, tag="ost", name="ost")
                oa = ps_o.tile([128, 1024], F32, tag="oa", bufs=1, name="oa")
                for oc in (0, 1):
                    for t in (0, 1):
                        nc.tensor.matmul(oa[:, oc * 512:(oc + 1) * 512],
                                         yP[t][:, mt * 128:(mt + 1) * 128],
                                         wpP_sb[t][:, oc * 512:(oc + 1) * 512],
                                         start=(t == 0), stop=(t == 1))
                nc.vector.tensor_copy(ost[:], oa[:])
                nc.sync.dma_start(out[mt * 128:(mt + 1) * 128, :], ost[:])

        # ---- emission ------------------------------------------------------
        with tc.tile_pool(name="psw", bufs=1, space="PSUM") as ps_w:
            st0 = ps_w.tile([16, 512], F32, tag="st", name="st0")
            for m in range(3):
                for ch in range(NCH):
                    proj_unit(ps_w, 0, m, ch, st0)
            lnexp(0, st0)

            # fin(p0) interleaved with P(p1)
            st1 = ps_w.tile([16, 512], F32, tag="st", name="st1")
            p1_units = [(m, ch) for m in range(3) for ch in range(NCH)]
            f0 = fin_chunks(0)
            fi = 0
            for i, (m, ch) in enumerate(p1_units):
                proj_unit(ps_w, 1, m, ch, st1)
                take = ((i + 1) * len(f0)) // len(p1_units) - fi
                for _ in range(take):
                    f0[fi]()
                    fi += 1
            lnexp(1, st1)

            # A(pair0) interleaved with fin(p1)
            f1 = fin_chunks(1)
            a0 = [(hh, ib) for hh in (0, 1) for ib in range(NIB)]
            fi = 0
            for i, (hh, ib) in enumerate(a0):
                take = ((i + 1) * len(f1)) // len(a0) - fi
                for _ in range(take):
                    f1[fi]()
                    fi += 1
                attn_unit(0, hh, ib)

        # pair1 attention + division + out-projection, per i-block
        with tc.tile_pool(name="pso", bufs=2, space="PSUM") as ps_o:
            for ib in range(NIB):
                attn_unit(1, 0, ib)
                attn_unit(1, 1, ib)
                ydiv_unit(ps_o, 0, ib)
                ydiv_unit(ps_o, 1, ib)
                o_unit(ps_o, ib)

        if debug_dump:
            for p in (0, 1):
                nc.sync.dma_start(dbg["d_qsb"][p], qsbF[p][0][:])
                nc.sync.dma_start(dbg["d_kT"][p], kT_sb[p][:])
                nc.sync.dma_start(dbg["d_yP"][p], yP[p][:])
            nc.sync.dma_start(dbg["d_sums"], sums_sb[:])
            for h in range(HL):
                nc.sync.dma_start(dbg["d_qTz"][h], qTz[h][:])
            nc.sync.dma_start(dbg["d_v"], v_sb[:])

    nc.compile()
    return nc


def _host_inputs(x, w_attn, w_proj):
    """Build the 8 per-core input maps (bf16 device tensors)."""
    bf = ml_dtypes.bfloat16
    inv_freq = 1.0 / (10000.0 ** (np.arange(0, D_HEAD, 2, dtype=np.float32)
                                  / D_HEAD))
    t = np.arange(T, dtype=np.float32)
    freqs = np.einsum('i,j->ij', t, inv_freq)            # [T, 32]
    emb = np.concatenate([freqs, freqs], 1)              # [T, 64]
    cos64 = np.cos(emb).T                                # [64, T]
    sin64 = np.sin(emb).T
    sgn = np.where(np.arange(64) < 32, -1.0, 1.0)[:, None].astype(np.float32)
    sinNeg64 = sin64 * sgn
    cosT = np.concatenate([cos64, cos64], 0).astype(bf)  # [128, T]
    sinNegT = np.concatenate([sinNeg64, sinNeg64], 0).astype(bf)

    tri = (np.arange(128)[:, None] <= np.arange(128)[None, :]).astype(bf)

    selqk = np.zeros((128, 8, 16), np.float32)
    selbc = np.zeros((16, 8, 128), np.float32)
    for m in range(2):
        for ch in range(NCH):
            u = m * 4 + ch
            for p in range(128):
                c = 4 * ch + 2 * m + (1 if p >= 64 else 0)
                selqk[p, u, c] = 1.0
                selbc[c, u, p] = 1.0
    sel4 = np.zeros((4, 2, 128), np.float32)
    for p in (0, 1):
        sel4[2 * p, p, 0:64] = 1.0
        sel4[2 * p + 1, p, 64:128] = 1.0

    zpadQ = np.zeros((64, T), np.float32)
    onescol = np.ones((128, HL * NTT), np.float32)

    wq = w_attn[:D_MODEL]
    wk = w_attn[D_MODEL:2 * D_MODEL]
    wv_full = w_attn[2 * D_MODEL:]

    in_maps = []
    for c in range(N_CORES):
        b, hg = c // 4, c % 4
        wA = np.zeros((2, D_MODEL, 384), np.float32)
        for hp in (0, 1):
            hs = slice((hg * 4 + 2 * hp) * D_HEAD,
                       (hg * 4 + 2 * hp + 2) * D_HEAD)     # 128 rows
            wA[hp, :, 0:128] = wq[hs].T
            wA[hp, :, 128:256] = wk[hs].T
            wA[hp, :, 256:384] = wv_full[hs].T
        wp_c = [w_proj[:, (hg * 4 + j) * D_HEAD:(hg * 4 + j + 1) * D_HEAD].T
                for j in range(HL)]
        wpP = np.stack([np.concatenate([wp_c[0], wp_c[1]], 0),
                        np.concatenate([wp_c[2], wp_c[3]], 0)])

        in_maps.append({
            "xT": np.ascontiguousarray(x[b].T).astype(bf),
            "wA": wA.astype(bf),
            "cosT": cosT, "sinNegT": sinNegT, "trimask": tri,
            "selqk": selqk.astype(bf), "selbc": selbc.astype(bf),
            "sel4": sel4.astype(bf),
            "zpadQ": zpadQ.astype(bf), "onescol": onescol.astype(bf),
            "wpP": wpP.astype(bf),
        })
    return in_maps


def kernel(x, w_attn, w_proj, _want_results=False):
    x = np.asarray(x, dtype=np.float32)
    w_attn = np.asarray(w_attn, dtype=np.float32)
    w_proj = np.asarray(w_proj, dtype=np.float32)

    if "nc" not in _cached:
        _cached["nc"] = _build()
    nc = _cached["nc"]

    in_maps = _host_inputs(x, w_attn, w_proj)
    res = run_bass_kernel_spmd(nc, in_maps, list(range(N_CORES)))

    full = np.zeros((B, T, D_MODEL), np.float32)
    for c in range(N_CORES):
        full[c // 4] += res.results[c]["out"]
    if _want_results:
        return full, res
    return full


# revision 42
# speedup vs baseline: 1.5146x; 1.1066x over previous
"""Causal self-attention (QK-RMSNorm + rotary, H=16, D=1024, B=2, T=2048) on 8 NeuronCores.

Sharding: core c handles batch b = c // 4 and heads 4*(c%4) .. 4*(c%4)+3,
processed as two head PAIRS. Each core computes the qkv projection for its
heads, causal attention, and a row-parallel slice of the output projection;
the host sums the 4 partial outputs per batch element.

v2 design (vs the fp32r baseline):
- All matmul operands in bf16 (x, weights, q, k, v, p, y): halves DMA/SBUF.
- Rotary via a cross-partition DMA shift (rot(q) = shift(q) * signed-sin)
  instead of a second full projection: saves ~57k PE cycles.
- RMS scale applied to raw q/k before rope (rope is norm-preserving and
  commutes with per-head scalars); stats batched into one ln+exp per pair.
- Attention: full-512-wide score matmuls, exp batched in [128,1024] pairs
  (amortizes ACT's per-instruction bubble), causal masking via a Pool-side
  tri multiply on the diagonal squares, y-matmuls restricted to [o:512].
- Softmax sums ride the v ones-column (partition 64/63 of yacc) as in the
  baseline; division is per i-block so the out-projection overlaps the tail
  of attention.
- Engine balance: PE does matmuls only; ACT does exps+stats; DVE does
  PSUM->SBUF copies and PSUM-operand muls; Pool (gpsimd) does SBUF-only
  muls/adds (rope combine, tri); DMA does the rotary shift and v transpose.
"""
import sys
sys.path.insert(0, '/opt/trn_rl_repo')

import numpy as np
import ml_dtypes
from contextlib import ExitStack

import concourse.bass as bass
import concourse.tile as tile
from concourse import bacc, mybir
from concourse.bass_utils import run_bass_kernel_spmd

F32 = mybir.dt.float32
BF = mybir.dt.bfloat16
AF = mybir.ActivationFunctionType

N_HEAD = 16
D_MODEL = 1024
D_HEAD = 64
B, T = 2, 2048
N_CORES = 8
HL = 4              # heads per core
KT = D_MODEL // 128  # 8 contraction tiles
NCH = T // 512      # 4 token chunks
NIB = T // 512      # 4 i-blocks
NTT = T // 128      # 16 j-tiles
SCALE = D_HEAD ** -0.5

_cached = {}


def _build(debug_dump=False):
    nc = bacc.Bacc("TRN2", target_bir_lowering=False, debug=False,
                   num_devices=N_CORES)

    # ---- DRAM I/O ----------------------------------------------------------
    xT = nc.dram_tensor("xT", [D_MODEL, T], BF, kind="ExternalInput").ap()
    wA = nc.dram_tensor("wA", [2, D_MODEL, 384], BF, kind="ExternalInput").ap()
    cosT = nc.dram_tensor("cosT", [128, T], BF, kind="ExternalInput").ap()
    sinNegT = nc.dram_tensor("sinNegT", [128, T], BF,
                             kind="ExternalInput").ap()
    trimask = nc.dram_tensor("trimask", [128, 128], BF,
                             kind="ExternalInput").ap()
    selqk = nc.dram_tensor("selqk", [128, 8, 16], BF,
                           kind="ExternalInput").ap()
    selbc = nc.dram_tensor("selbc", [16, 8, 128], BF,
                           kind="ExternalInput").ap()
    sel4 = nc.dram_tensor("sel4", [4, 2, 128], BF, kind="ExternalInput").ap()
    zpadQ = nc.dram_tensor("zpadQ", [64, T], BF, kind="ExternalInput").ap()
    onescol = nc.dram_tensor("onescol", [128, HL * NTT], BF,
                             kind="ExternalInput").ap()
    wpP = nc.dram_tensor("wpP", [2, 128, 1024], BF, kind="ExternalInput").ap()
    out = nc.dram_tensor("out", [T, D_MODEL], F32, kind="ExternalOutput").ap()
    if debug_dump:
        dbg = {
            "d_qsb": nc.dram_tensor("d_qsb", [2, 128, T], BF,
                                    kind="ExternalOutput").ap(),
            "d_qTz": nc.dram_tensor("d_qTz", [HL, 128, T], BF,
                                    kind="ExternalOutput").ap(),
            "d_kT": nc.dram_tensor("d_kT", [2, 128, T], BF,
                                   kind="ExternalOutput").ap(),
            "d_v": nc.dram_tensor("d_v", [128, 5200], BF,
                                  kind="ExternalOutput").ap(),
            "d_sums": nc.dram_tensor("d_sums", [4, NIB, 512], F32,
                                     kind="ExternalOutput").ap(),
            "d_yP": nc.dram_tensor("d_yP", [2, 128, T], BF,
                                   kind="ExternalOutput").ap(),
        }

    with tile.TileContext(nc) as tc, ExitStack() as ctx:
        ctx.enter_context(nc.allow_low_precision(
            reason="bf16 matmuls/intermediates; tolerance is 2e-2"))

        cpool = ctx.enter_context(tc.tile_pool(name="consts", bufs=1))
        work = ctx.enter_context(tc.tile_pool(name="work", bufs=2))
        ps_s = ctx.enter_context(tc.tile_pool(name="pss", bufs=2,
                                              space="PSUM"))
        ps_y = ctx.enter_context(tc.tile_pool(name="psy", bufs=1,
                                              space="PSUM"))

        # ---- persistent SBUF -----------------------------------------------
        x_sb = cpool.tile([128, KT, T], BF)
        wA_sb = [cpool.tile([128, KT, 384], BF, name=f"wA{p}") for p in (0, 1)]
        cos_sb = cpool.tile([128, T], BF)
        sinNeg_sb = cpool.tile([128, T], BF)
        tri_sb = cpool.tile([128, 128], BF)
        selqk_sb = cpool.tile([128, 8, 16], BF)
        selbc_sb = cpool.tile([16, 8, 128], BF)
        sel4_sb = cpool.tile([4, 2, 128], BF)
        wpP_sb = [cpool.tile([128, 1024], BF, name=f"wpP{p}") for p in (0, 1)]
        qTz = [cpool.tile([128, T], BF, name=f"qTz{h}") for h in range(HL)]
        kT_sb = [cpool.tile([128, T], BF, name=f"kT{p}") for p in (0, 1)]
        # padded past HL*NTT*65+64 so the [p, 2, 1040] transpose-dst view of
        # the last head pair stays in bounds
        v_sb = cpool.tile([128, 5200], BF)
        v3 = v_sb[:, 0:HL * NTT * 65].rearrange("p (g o) -> p g o", o=65)
        yP = [cpool.tile([128, T], BF, name=f"yP{p}") for p in (0, 1)]
        qsbF = [[cpool.tile([128, T], BF, name=f"qsb{p}{m}") for m in (0, 1)]
                for p in (0, 1)]
        qshF = [[cpool.tile([128, T], BF, name=f"qsh{p}{m}") for m in (0, 1)]
                for p in (0, 1)]
        rinv_sb = [cpool.tile([16, 512], BF, name=f"rinv{p}") for p in (0, 1)]
        sums_sb = cpool.tile([4, NIB, 512], F32)
        rinvy_sb = cpool.tile([4, NIB, 512], BF)

        # ---- preamble DMAs -------------------------------------------------
        nc.sync.dma_start(wA_sb[0][:],
                          wA[0].rearrange("(k p) c -> p k c", p=128))
        for ch in range(NCH):
            nc.sync.dma_start(x_sb[:, :, ch * 512:(ch + 1) * 512],
                              xT.rearrange("(k p) t -> p k t", p=128)
                              [:, :, ch * 512:(ch + 1) * 512])
        nc.sync.dma_start(selqk_sb[:], selqk[:])
        nc.sync.dma_start(wA_sb[1][:],
                          wA[1].rearrange("(k p) c -> p k c", p=128))
        nc.sync.dma_start(cos_sb[:], cosT[:])
        nc.sync.dma_start(sinNeg_sb[:], sinNegT[:])
        nc.sync.dma_start(tri_sb[:], trimask[:])
        nc.sync.dma_start(selbc_sb[:], selbc[:])
        nc.sync.dma_start(sel4_sb[:], sel4[:])
        for h in range(HL):
            half = slice(64, 128) if h % 2 == 0 else slice(0, 64)
            nc.sync.dma_start(qTz[h][half, :], zpadQ[:])
        nc.sync.dma_start(v3[:, :, 64:65], onescol.unsqueeze(2))
        for p in (0, 1):
            nc.sync.dma_start(wpP_sb[p][:], wpP[p])

        ySG_store = {}

        # ---- unit emitters -------------------------------------------------
        def proj_unit(ps_w, hp, m, ch, st_tile):
            cs = slice(ch * 512, (ch + 1) * 512)
            acc = ps_w.tile([128, 512], F32, tag="pa", bufs=2, name="acc")
            for k in range(KT):
                nc.tensor.matmul(acc[:],
                                 wA_sb[hp][:, k, m * 128:(m + 1) * 128],
                                 x_sb[:, k, cs], start=(k == 0),
                                 stop=(k == KT - 1))
            if m < 2:
                dst = qsbF[hp][m][:, cs]
                nc.scalar.copy(dst, acc[:])
                if ch == NCH - 1:
                    # all 4 chunks landed: batched square + 4 stats matmuls
                    sqF = qshF[hp][m]
                    nc.vector.tensor_mul(sqF[:], qsbF[hp][m][:],
                                         qsbF[hp][m][:])
                    for c2 in range(NCH):
                        idx = m * 4 + c2
                        nc.tensor.matmul(
                            st_tile[:], selqk_sb[:, idx, :],
                            sqF[:, c2 * 512:(c2 + 1) * 512],
                            start=(idx == 0), stop=(idx == 7))
            else:
                vdst = work.tile([128, 512], BF, tag="vsb", bufs=3,
                                 name="vdst")
                nc.vector.tensor_copy(vdst[:], acc[:])
                for s4 in range(4):
                    jt = ch * 4 + s4
                    gA = (2 * hp) * NTT + jt
                    vstg = work.tile([128, 128], BF, tag="vstg", bufs=3,
                                     name="vstg")
                    nc.sync.dma_start_transpose(
                        vstg[:], vdst[:, s4 * 128:(s4 + 1) * 128])
                    vv = v_sb[:, gA * 65:gA * 65 + 2080] \
                        .rearrange("p (h x) -> p h x", h=2)[:, :, 0:64]
                    nc.gpsimd.tensor_copy(
                        vv, vstg[:].rearrange("p (h x) -> p h x", h=2))

        def lnexp(hp, st_tile):
            lnt = work.tile([16, 512], F32, tag="lnt", bufs=1, name="lnt")
            nc.scalar.activation(lnt[:], st_tile[:], AF.Ln, scale=1.0 / 64.0)
            nc.scalar.activation(rinv_sb[hp][:], lnt[:], AF.Exp, scale=-0.5)

        def fin_chunks(hp):
            """Chunk closures: rms-scale in place, rotary shift + combine.

            Full-T granularity: t1 overwrites qsbF, t2 overwrites qshF
            (both dead afterwards), so no scratch tiles are needed."""
            chunks = []
            for m in (0, 1):
                for ch in range(NCH):
                    def bc_scale(m=m, ch=ch):
                        cs = slice(ch * 512, (ch + 1) * 512)
                        bc = ps_s.tile([128, 512], F32, tag="s", name="bc")
                        nc.tensor.matmul(bc[:], selbc_sb[:, m * 4 + ch, :],
                                         rinv_sb[hp][:], start=True,
                                         stop=True)
                        nc.vector.tensor_mul(qsbF[hp][m][:, cs],
                                             qsbF[hp][m][:, cs], bc[:])
                    chunks.append(bc_scale)

                def shifts(m=m):
                    for blk in range(4):
                        d0 = blk * 32
                        s0 = (blk ^ 1) * 32
                        nc.sync.dma_start(qshF[hp][m][d0:d0 + 32, :],
                                          qsbF[hp][m][s0:s0 + 32, :])
                chunks.append(shifts)

            def rope_mul(m, which):
                if which == 0:
                    nc.vector.tensor_mul(qsbF[hp][m][:], qsbF[hp][m][:],
                                         cos_sb[:])
                else:
                    nc.vector.tensor_mul(qshF[hp][m][:], qshF[hp][m][:],
                                         sinNeg_sb[:])

            def rope_add(m):
                if m == 0:
                    nc.vector.tensor_add(qTz[2 * hp][0:64, :],
                                         qsbF[hp][0][0:64, :],
                                         qshF[hp][0][0:64, :])
                    nc.vector.tensor_add(qTz[2 * hp + 1][64:128, :],
                                         qsbF[hp][0][64:128, :],
                                         qshF[hp][0][64:128, :])
                else:
                    nc.vector.tensor_add(kT_sb[hp][:], qsbF[hp][1][:],
                                         qshF[hp][1][:])

            chunks.append(lambda: rope_mul(0, 1))
            chunks.append(lambda: rope_mul(1, 1))
            chunks.append(lambda: rope_mul(0, 0))
            chunks.append(lambda: rope_add(0))
            chunks.append(lambda: rope_mul(1, 0))
            chunks.append(lambda: rope_add(1))
            return chunks

        def attn_unit(hp, hh, ib):
            h_l = 2 * hp + hh
            njt = 4 * (ib + 1)
            ibs = ib * 512
            yacc = ps_y.tile([128, 512], F32, tag="y", name="yacc")

            def ymms(pr, pt):
                for half in (0, 1):
                    jt = 2 * pr + half
                    o = max(0, jt * 128 - ibs)
                    g = h_l * NTT + jt
                    if hh == 0:
                        vau = v_sb[:, g * 65:g * 65 + 128]
                    else:
                        vau = v_sb[:, g * 65 - 64:g * 65 + 64]
                    nc.tensor.matmul(yacc[:, o:512], vau,
                                     pt[:, half * 512 + o:half * 512 + 512],
                                     start=(jt == 0), stop=(jt == njt - 1))

            prev = None
            for pr in range(njt // 2):
                sp = ps_s.tile([128, 1024], F32, tag="s", name="sp")
                for half in (0, 1):
                    jt = 2 * pr + half
                    nc.tensor.matmul(sp[:, half * 512:(half + 1) * 512],
                                     kT_sb[hp][:, jt * 128:(jt + 1) * 128],
                                     qTz[h_l][:, ibs:ibs + 512],
                                     start=True, stop=True)
                pt = work.tile([128, 1024], BF, tag="p", bufs=3, name="pt")
                nc.scalar.activation(pt[:], sp[:], AF.Exp, scale=SCALE)
                for half in (0, 1):
                    jt = 2 * pr + half
                    o = jt * 128 - ibs
                    if o >= 0:
                        lo = half * 512 + o
                        nc.vector.tensor_mul(pt[:, lo:lo + 128],
                                             pt[:, lo:lo + 128], tri_sb[:])
                if prev is not None:
                    ymms(*prev)
                prev = (pr, pt)
            ymms(*prev)

            ySG = work.tile([128, 512], F32, tag="ysg", bufs=12, name="ySG")
            nc.vector.tensor_copy(ySG[:], yacc[:])
            srow = 64 if hh == 0 else 63
            nc.sync.dma_start(sums_sb[2 * hp + hh:2 * hp + hh + 1, ib, :],
                              ySG[srow:srow + 1, :])
            ySG_store[(hp, hh, ib)] = ySG

        def ydiv_unit(ps_o, hp, ib):
            ibs = ib * 512
            if hp == 0:
                # one reciprocal per i-block covers both pairs (cost is
                # free-size-dominated, so batching partitions is free)
                nc.vector.reciprocal(rinvy_sb[:, ib, :], sums_sb[:, ib, :])
            bc2 = ps_o.tile([128, 512], F32, tag="bc2", bufs=1, name="bc2")
            nc.tensor.matmul(bc2[:], sel4_sb[:, hp, :], rinvy_sb[:, ib, :],
                             start=True, stop=True)
            e = ySG_store[(hp, 0, ib)]
            o_ = ySG_store[(hp, 1, ib)]
            nc.vector.tensor_mul(yP[hp][0:64, ibs:ibs + 512], e[0:64, :],
                                 bc2[0:64, :])
            nc.vector.tensor_mul(yP[hp][64:128, ibs:ibs + 512],
                                 o_[64:128, :], bc2[64:128, :])

        def o_unit(ps_o, ib):
            for mt in range(4 * ib, 4 * ib + 4):
                ost = work.tile([128, 1024], F32, tag="ost", name="ost")
                oa = ps_o.tile([128, 1024], F32, tag="oa", bufs=1, name="oa")
                for oc in (0, 1):
                    for t in (0, 1):
                        nc.tensor.matmul(oa[:, oc * 512:(oc + 1) * 512],
                                         yP[t][:, mt * 128:(mt + 1) * 128],
                                         wpP_sb[t][:, oc * 512:(oc + 1) * 512],
                                         start=(t == 0), stop=(t == 1))
                nc.vector.tensor_copy(ost[:], oa[:])
                nc.sync.dma_start(out[mt * 128:(mt + 1) * 128, :], ost[:])

        # ---- emission ------------------------------------------------------
        with tc.tile_pool(name="psw", bufs=1, space="PSUM") as ps_w:
            st0 = ps_w.tile([16, 512], F32, tag="st", name="st0")
            for m in range(3):
                for ch in range(NCH):
                    proj_unit(ps_w, 0, m, ch, st0)
            lnexp(0, st0)

            # fin(p0) interleaved with P(p1)
            st1 = ps_w.tile([16, 512], F32, tag="st", name="st1")
            p1_units = [(m, ch) for m in range(3) for ch in range(NCH)]
            f0 = fin_chunks(0)
            fi = 0
            for i, (m, ch) in enumerate(p1_units):
                proj_unit(ps_w, 1, m, ch, st1)
                take = ((i + 1) * len(f0)) // len(p1_units) - fi
                for _ in range(take):
                    f0[fi]()
                    fi += 1
            lnexp(1, st1)

            # A(pair0) interleaved with fin(p1)
            f1 = fin_chunks(1)
            a0 = [(hh, ib) for hh in (0, 1) for ib in range(NIB)]
            fi = 0
            for i, (hh, ib) in enumerate(a0):
                take = ((i + 1) * len(f1)) // len(a0) - fi
                for _ in range(take):
                    f1[fi]()
                    fi += 1
                attn_unit(0, hh, ib)

        # pair1 attention + division + out-projection, per i-block
        with tc.tile_pool(name="pso", bufs=2, space="PSUM") as ps_o:
            for ib in range(NIB):
                attn_unit(1, 0, ib)
                attn_unit(1, 1, ib)
                if ib >= 1:
                    ydiv_unit(ps_o, 0, ib - 1)
                    ydiv_unit(ps_o, 1, ib - 1)
                    o_unit(ps_o, ib - 1)
            ydiv_unit(ps_o, 0, NIB - 1)
            ydiv_unit(ps_o, 1, NIB - 1)
            o_unit(ps_o, NIB - 1)

        if debug_dump:
            for p in (0, 1):
                nc.sync.dma_start(dbg["d_qsb"][p], qsbF[p][0][:])
                nc.sync.dma_start(dbg["d_kT"][p], kT_sb[p][:])
                nc.sync.dma_start(dbg["d_yP"][p], yP[p][:])
            nc.sync.dma_start(dbg["d_sums"], sums_sb[:])
            for h in range(HL):
                nc.sync.dma_start(dbg["d_qTz"][h], qTz[h][:])
            nc.sync.dma_start(dbg["d_v"], v_sb[:])

    nc.compile()
    return nc


def _host_inputs(x, w_attn, w_proj):
    """Build the 8 per-core input maps (bf16 device tensors)."""
    bf = ml_dtypes.bfloat16
    inv_freq = 1.0 / (10000.0 ** (np.arange(0, D_HEAD, 2, dtype=np.float32)
                                  / D_HEAD))
    t = np.arange(T, dtype=np.float32)
    freqs = np.einsum('i,j->ij', t, inv_freq)            # [T, 32]
    emb = np.concatenate([freqs, freqs], 1)              # [T, 64]
    cos64 = np.cos(emb).T                                # [64, T]
    sin64 = np.sin(emb).T
    sgn = np.where(np.arange(64) < 32, -1.0, 1.0)[:, None].astype(np.float32)
    sinNeg64 = sin64 * sgn
    cosT = np.concatenate([cos64, cos64], 0).astype(bf)  # [128, T]
    sinNegT = np.concatenate([sinNeg64, sinNeg64], 0).astype(bf)

    tri = (np.arange(128)[:, None] <= np.arange(128)[None, :]).astype(bf)

    selqk = np.zeros((128, 8, 16), np.float32)
    selbc = np.zeros((16, 8, 128), np.float32)
    for m in range(2):
        for ch in range(NCH):
            u = m * 4 + ch
            for p in range(128):
                c = 4 * ch + 2 * m + (1 if p >= 64 else 0)
                selqk[p, u, c] = 1.0
                selbc[c, u, p] = 1.0
    sel4 = np.zeros((4, 2, 128), np.float32)
    for p in (0, 1):
        sel4[2 * p, p, 0:64] = 1.0
        sel4[2 * p + 1, p, 64:128] = 1.0

    zpadQ = np.zeros((64, T), np.float32)
    onescol = np.ones((128, HL * NTT), np.float32)

    wq = w_attn[:D_MODEL]
    wk = w_attn[D_MODEL:2 * D_MODEL]
    wv_full = w_attn[2 * D_MODEL:]

    in_maps = []
    for c in range(N_CORES):
        b, hg = c // 4, c % 4
        wA = np.zeros((2, D_MODEL, 384), np.float32)
        for hp in (0, 1):
            hs = slice((hg * 4 + 2 * hp) * D_HEAD,
                       (hg * 4 + 2 * hp + 2) * D_HEAD)     # 128 rows
            wA[hp, :, 0:128] = wq[hs].T
            wA[hp, :, 128:256] = wk[hs].T
            wA[hp, :, 256:384] = wv_full[hs].T
        wp_c = [w_proj[:, (hg * 4 + j) * D_HEAD:(hg * 4 + j + 1) * D_HEAD].T
                for j in range(HL)]
        wpP = np.stack([np.concatenate([wp_c[0], wp_c[1]], 0),
                        np.concatenate([wp_c[2], wp_c[3]], 0)])

        in_maps.append({
            "xT": np.ascontiguousarray(x[b].T).astype(bf),
            "wA": wA.astype(bf),
            "cosT": cosT, "sinNegT": sinNegT, "trimask": tri,
            "selqk": selqk.astype(bf), "selbc": selbc.astype(bf),
            "sel4": sel4.astype(bf),
            "zpadQ": zpadQ.astype(bf), "onescol": onescol.astype(bf),
            "wpP": wpP.astype(bf),
        })
    return in_maps


def kernel(x, w_attn, w_proj, _want_results=False):
    x = np.asarray(x, dtype=np.float32)
    w_attn = np.asarray(w_attn, dtype=np.float32)
    w_proj = np.asarray(w_proj, dtype=np.float32)

    if "nc" not in _cached:
        _cached["nc"] = _build()
    nc = _cached["nc"]

    in_maps = _host_inputs(x, w_attn, w_proj)
    res = run_bass_kernel_spmd(nc, in_maps, list(range(N_CORES)))

    full = np.zeros((B, T, D_MODEL), np.float32)
    for c in range(N_CORES):
        full[c // 4] += res.results[c]["out"]
    if _want_results:
        return full, res
    return full
